# revision 23
# baseline (speedup 1.0000x reference)
"""GATNet (4-layer GAT, 10000 nodes / 50000 edges + self-loops) on 8 Trainium2 NeuronCores.

Self-contained: builds per-core shards on the host (edge bucketing by destination,
one-hot scatter masks, gather index tables), compiles one SPMD Bass program, runs it
on cores 0-7 via run_bass_kernel_spmd, and reassembles the full [10000, 1000] output.

Structure per layer:
  dense h = y @ W^T (bf16, attention projections folded as extra rhs columns).
  The previous layer's graph-LN is folded in: diag(lnw) is baked into the rhs
  weights on the host; the runtime rinv scalar and the (lnb - mu*rinv*lnw)@W
  row const (broadcast via a K=1 matmul) are applied at the PSUM->SBUF
  eviction on the Vector engine, so the h table holds the exact LN'd h and
  the dense consumes raw (pre-LN) zT -- no LN apply pass, no stats on the
  dense critical path.
  -> small AllGather of the per-node attention scores (al); the softmax
     pre-phase (als gathers; ald via one-hot maskT matmuls from the local al
     shard; e-values, segment denominators, reciprocals, alphas) overlaps the
     dense; the two big half-table AllGathers are emitted after it (collective
     instructions block the gpsimd queue until completion, so the pre-phase
     gathers must be issued first)
  -> gather phase: dma_gather of source h rows; one-hot scatter matmuls with
     the per-edge alpha folded into the stationary operand; bias via K=1
     matmul; ReLU eviction (ACT, fused row sums) with all PSUM->SBUF copies on
     the Vector engine; PE-transpose into feature-major for the next lhsT
  -> graph-LN stats via tiny AllReduce -> (mu, rinv) broadcast + next biasT.
Layer 1 never materializes h: by linearity sum_e alpha_e * (x W)[src_e] =
(sum_e alpha_e x[src_e]) W, so it scatters raw x rows (256 wide) and applies W1
once per destination window; its attention scores are computed exactly on the host.
"""
import sys
import types

import numpy as np
import ml_dtypes

BF16 = ml_dtypes.bfloat16

N_NODES = 10000
N_CORES = 8
NPC = 1250
NPAD = 1280
NT = 10
NW = 10
HEADS_L = [8, 8, 8, 1]
C_L = [448, 384, 256, 1000]
FIN_L = [256, 3584, 3072, 2048]
FOUT_L = [3584, 3072, 2048, 1000]
TCOL_L = [256, 3072, 2048, 1024]    # bf16 columns of the gather table (L1: raw x)
EXP_CLAMP = 35.0
DEN_TINY = 1e-30


def _install_ntff_hook():
    if "antenv.axon_hooks" in sys.modules:
        return
    try:
        import antenv
        from trn_agent_boot.trn_boot import _ntff_profile_via_ctypes
    except ImportError:
        return
    mod = types.ModuleType("antenv.axon_hooks")
    state = {"hook": None}
    mod.set_axon_ntff_profile_hook = lambda h: state.__setitem__("hook", h)
    mod.get_axon_ntff_profile_hook = lambda: state["hook"]
    sys.modules["antenv.axon_hooks"] = mod
    antenv.axon_hooks = mod
    mod.set_axon_ntff_profile_hook(_ntff_profile_via_ctypes("/opt/axon/libaxon_pjrt.so"))


# ---------------------------------------------------------------- host prep
def _table_row(n):
    return NPAD * (n // NPC) + (n % NPC)


def _wrap16(idx_chunk):
    w = idx_chunk.reshape(8, 16).T
    return np.tile(w, (8, 1)).astype(np.int16)


def prep_edges(edge_index):
    src = np.asarray(edge_index[0], dtype=np.int64)
    dst = np.asarray(edge_index[1], dtype=np.int64)
    src = np.concatenate([src, np.arange(N_NODES, dtype=np.int64)])
    dst = np.concatenate([dst, np.arange(N_NODES, dtype=np.int64)])

    buckets = [[[] for _ in range(NW)] for _ in range(N_CORES)]
    core_of = dst // NPC
    win_of = (dst % NPC) // 128
    order = np.argsort(dst, kind="stable")
    for e in order:
        buckets[core_of[e]][win_of[e]].append(e)

    ncw = []
    for w in range(NW):
        mx = max(len(buckets[k][w]) for k in range(N_CORES))
        ncw.append(max(1, -(-mx // 128)))
    nch = sum(ncw)

    per_core = []
    for k in range(N_CORES):
        idx_s = np.zeros((128, nch * 8), np.int16)
        idx_d = np.zeros((128, nch * 8), np.int16)
        mask = np.zeros((128, nch, 128), np.float32)
        maskT = np.zeros((128, nch, 128), np.float32)
        esrc = np.zeros((nch, 128), np.int64)
        edst = np.zeros((nch, 128), np.int64)
        ereal = np.zeros((nch, 128), bool)
        c0 = 0
        for w in range(NW):
            edges = buckets[k][w]
            for c in range(ncw[w]):
                part = edges[c * 128:(c + 1) * 128]
                srows = np.zeros(128, np.int64)
                drows = np.zeros(128, np.int64)
                for i, e in enumerate(part):
                    srows[i] = _table_row(src[e])
                    drows[i] = _table_row(dst[e])
                    esrc[c0 + c, i] = src[e]
                    edst[c0 + c, i] = dst[e]
                    ereal[c0 + c, i] = True
                    d_local = (dst[e] % NPC) - 128 * w
                    mask[i, c0 + c, d_local] = 1.0
                    maskT[d_local, c0 + c, i] = 1.0
                idx_s[:, (c0 + c) * 8:(c0 + c + 1) * 8] = _wrap16(srows)
                idx_d[:, (c0 + c) * 8:(c0 + c + 1) * 8] = _wrap16(drows)
            c0 += ncw[w]
        per_core.append(dict(
            idxs=idx_s,
            mask=mask.reshape(128, nch * 128).astype(BF16),
            maskT=maskT.reshape(128, nch * 128).astype(BF16),
            _esrc=esrc, _edst=edst, _ereal=ereal,
        ))
    return tuple(ncw), per_core


def prep_alpha1(per_core, ncw, al1):
    """Exact layer-1 softmax on the host: alpha[e, h] per (chunk, slot)."""
    nch = sum(ncw)
    als = al1[:, :8].astype(np.float64)
    ald = al1[:, 8:].astype(np.float64)
    out = []
    for pc in per_core:
        esrc, edst, ereal = pc["_esrc"], pc["_edst"], pc["_ereal"]
        e = als[esrc] + ald[edst]                      # [nch, 128, 8]
        e = np.maximum(e, 0.2 * e)
        wv = np.exp(np.minimum(e, EXP_CLAMP)) * ereal[:, :, None]
        den = np.zeros((NPC, 8))
        np.add.at(den, (edst % NPC).reshape(-1), wv.reshape(-1, 8))
        alpha = wv / np.maximum(den[(edst % NPC)], 1e-300)
        # device layout: [part=slot, nch*8]
        return_arr = np.ascontiguousarray(
            alpha.transpose(1, 0, 2).reshape(128, nch * 8)).astype(BF16)
        out.append(return_arr)
    return out


def prep_params(inputs):
    p = {}
    x64 = np.asarray(inputs["x"], np.float64)
    al1 = None
    for li in range(4):
        H, C, fin, fout = HEADS_L[li], C_L[li], FIN_L[li], FOUT_L[li]
        W = np.asarray(inputs[f"W{li+1}"], np.float64)
        a_src = np.asarray(inputs[f"a_src{li+1}"], np.float64)
        a_dst = np.asarray(inputs[f"a_dst{li+1}"], np.float64)
        a_blk_s = np.zeros((fout, H), np.float64)
        a_blk_d = np.zeros((fout, H), np.float64)
        for h in range(H):
            a_blk_s[h * C:(h + 1) * C, h] = a_src[h]
            a_blk_d[h * C:(h + 1) * C, h] = a_dst[h]
        rhs = np.concatenate([W.T, W.T @ a_blk_s, W.T @ a_blk_d], axis=1)
        # graph-LN of the previous layer folded in: y = rinv*(z*lnw) + (lnb - mu*rinv*lnw)
        # so h = y@rhs = rinv*(z @ diag(lnw)@rhs) + r1 - (mu*rinv)*r2 with the
        # static diag(lnw) baked into the device rhs and r1/r2 host rows.
        if li > 0:
            lwp = np.asarray(inputs[f"ln{li}_w"], np.float64)
            lbp = np.asarray(inputs[f"ln{li}_b"], np.float64)
            p[f"rhs{li+1}"] = np.ascontiguousarray(lwp[:, None] * rhs).astype(BF16)
            p[f"r1_{li+1}"] = (lbp @ rhs).reshape(1, fout + 2 * H).astype(np.float32)
            p[f"r2_{li+1}"] = (lwp @ rhs).reshape(1, fout + 2 * H).astype(np.float32)
        else:
            p[f"rhs{li+1}"] = np.ascontiguousarray(rhs).astype(BF16)
        p[f"brow{li+1}"] = np.asarray(inputs[f"b{li+1}"], np.float32).reshape(1, fout).astype(BF16)
        if li == 0:
            al1 = np.concatenate(
                [x64 @ (W.T @ a_blk_s), x64 @ (W.T @ a_blk_d)], axis=1).astype(np.float32)
    # packed x table, replicated: [10240 rows, 768 bytes] = 512B x bf16 + 256B f32 al
    xtab = np.zeros((N_CORES * NPAD, 768), np.uint8)
    rows = _table_row(np.arange(N_NODES))
    xb = np.asarray(inputs["x"], np.float32).astype(BF16)
    xtab[rows, :512] = xb.view(np.uint8)
    alpad = np.zeros((N_NODES, 64), np.float32)
    alpad[:, :16] = al1
    xtab[rows, 512:768] = alpad.view(np.uint8)
    p["xtab"] = xtab.view(BF16)
    p["_al1"] = al1
    p["ident"] = np.eye(128, dtype=BF16)
    ones_b = np.zeros((1, NW * 128), np.float32)
    ones_b[0, :NPC] = 1.0
    p["ones_b"] = ones_b.astype(BF16)
    return p


# ---------------------------------------------------------------- device build
_CACHE = {}


def build(ncw, debug=False):
    key = (tuple(ncw), debug)
    if key in _CACHE:
        return _CACHE[key]

    import concourse.bacc as bacc
    import concourse.mybir as mybir
    import concourse.tile as tile
    from concourse.library_config import mlp

    f32 = mybir.dt.float32
    bf16 = mybir.dt.bfloat16
    i16 = mybir.dt.int16
    AX = mybir.AxisListType
    ALU = mybir.AluOpType
    ACTF = mybir.ActivationFunctionType

    nch = sum(ncw)
    ncmax = max(ncw)
    coff = [0]
    for w in range(NW):
        coff.append(coff[-1] + ncw[w])
    nc = bacc.Bacc("TRN2", num_swdge_queues=4)

    xtab_d = nc.declare_dram_parameter("xtab", [N_CORES * NPAD, 384], bf16, isOutput=False)
    rhs_d, brow_d, r1_d, r2_d = [], [], [None], [None]
    for li in range(4):
        H, fout, fin = HEADS_L[li], FOUT_L[li], FIN_L[li]
        rhs_d.append(nc.declare_dram_parameter(f"rhs{li+1}", [fin, fout + 2 * H], bf16, isOutput=False))
        brow_d.append(nc.declare_dram_parameter(f"brow{li+1}", [1, fout], bf16, isOutput=False))
        if li > 0:
            r1_d.append(nc.declare_dram_parameter(f"r1_{li+1}", [1, fout + 2 * H], f32, isOutput=False))
            r2_d.append(nc.declare_dram_parameter(f"r2_{li+1}", [1, fout + 2 * H], f32, isOutput=False))
    alpha1_d = nc.declare_dram_parameter("alpha1", [128, nch * 8], bf16, isOutput=False)
    idxs_d = nc.declare_dram_parameter("idxs", [128, nch * 8], i16, isOutput=False)
    mask_d = nc.declare_dram_parameter("mask", [128, nch * 128], bf16, isOutput=False)
    maskT_d = nc.declare_dram_parameter("maskT", [128, nch * 128], bf16, isOutput=False)
    ones_d = nc.declare_dram_parameter("ones_b", [1, NW * 128], bf16, isOutput=False)
    ident_d = nc.declare_dram_parameter("ident", [128, 128], bf16, isOutput=False)
    out_d = nc.declare_dram_parameter("out", [NPC, 1000], f32, isOutput=True)
    dbg_zt, dbg_st = [], []
    if debug:
        for li in range(3):
            dbg_zt.append(nc.declare_dram_parameter(
                f"dbg_zt{li}", [128, 28 * NPAD], bf16, isOutput=True))
            dbg_st.append(nc.declare_dram_parameter(
                f"dbg_st{li}", [1, 8], f32, isOutput=True))

    RG = [list(range(N_CORES))]

    with tile.TileContext(nc) as tc:
        with (
            tc.tile_pool(name="const", bufs=1) as constp,
            tc.tile_pool(name="yt", bufs=1) as ytp,
            tc.tile_pool(name="rhs", bufs=5) as rhsp,
            tc.tile_pool(name="stage", bufs=2) as stagep,
            tc.tile_pool(name="gath", bufs=2) as gathp,
            tc.tile_pool(name="mw", bufs=2) as mwp,
            tc.tile_pool(name="eph", bufs=2) as ephp,
            tc.tile_pool(name="z", bufs=2) as zp,
            tc.tile_pool(name="misc", bufs=2) as miscp,
            tc.tile_pool(name="dram", bufs=1, space="DRAM") as dram,
        ):
            nc.gpsimd.load_library(mlp)

            idxs_t = constp.tile([128, nch, 8], i16, tag="idxs")
            nc.sync.dma_start(idxs_t[:], idxs_d[:].rearrange("p (c d) -> p c d", c=nch))
            ones_t = constp.tile([1, NW, 128], bf16, tag="onesb")
            nc.sync.dma_start(ones_t[:], ones_d[:].rearrange("p (w d) -> p w d", w=NW))
            ident_t = constp.tile([128, 128], bf16, tag="ident")
            nc.sync.dma_start(ident_t[:], ident_d[:])
            ones128 = constp.tile([128, 1], f32, tag="ones128")
            nc.vector.memset(ones128[:], 1.0)
            onesT = constp.tile([1, 128], f32, tag="onesT")
            nc.vector.memset(onesT[:], 1.0)
            alpha1_t = constp.tile([128, nch, 8], bf16, tag="alpha1")
            nc.sync.dma_start(alpha1_t[:], alpha1_d[:].rearrange("p (c d) -> p c d", c=nch))
            # resident W1 rhs (small; needed per destination window in layer 1)
            rt1 = constp.tile([128, 2, 3584], bf16, tag="rt1")
            nc.sync.dma_start(
                rt1[:], rhs_d[0][:, 0:3584].rearrange("(k p) n -> p k n", p=128))

            yT = None  # produced by each layer's LN for the next layer
            pending_biasT = []  # deferred biasT chunk builders for the next dense

            biasT = None  # [128, fout+2H] f32: rinv-independent LN row consts for this dense
            br = None     # [128, 2] f32: (mu, rinv) of previous layer's graph-LN

            for li in range(4):
                H, C, fin, fout = HEADS_L[li], C_L[li], FIN_L[li], FOUT_L[li]
                tcol = TCOL_L[li]
                kch = fin // 128
                acols = 2 * H

                brow = constp.tile([1, 3584], bf16, tag="brow")
                nc.sync.dma_start(brow[:, :fout], brow_d[li][:])

                if li > 0:
                    # ===== dense: h = y @ W^T (+ al columns); al chunk FIRST so the
                    # small al AllGather + softmax pre-phase overlap the dense phase
                    half = tcol // 2
                    shard_a = dram.tile([NPAD, half], bf16, tag=f"sharda{li}")
                    shard_b = dram.tile([NPAD, tcol - half], bf16, tag=f"shardb{li}")
                    glob_a = dram.tile([N_CORES * NPAD, half], bf16, addr_space="Shared", tag=f"globa{li}")
                    glob_b = dram.tile([N_CORES * NPAD, tcol - half], bf16, addr_space="Shared", tag=f"globb{li}")
                    ashard = dram.tile([NPAD, 64], f32, tag=f"ashard{li}")
                    aglob = dram.tile([N_CORES * NPAD, 64], f32, addr_space="Shared", tag=f"aglob{li}")
                    fcs = [(fout, acols)]
                    o = 0
                    while o < fout:
                        w_ = min(512, fout - o)
                        fcs.append((o, w_))
                        o += w_
                    # fci index after which all shard_a columns are written
                    fci_a_done = 0
                    o = 0
                    for fci, (fo, fw) in enumerate(fcs):
                        if fci > 0 and fo + fw <= half:
                            fci_a_done = fci
                    with tc.tile_pool(name=f"psA{li}", bufs=4, space="PSUM") as mmp:
                        # ---- softmax pre-phase, emitted as pipelined stages
                        # interleaved between dense row-tiles so PE/vector/
                        # scalar work overlaps the dense instead of trailing it
                        al_f32 = aglob[:]
                        alpha_all = ephp.tile([128, nch, 8], bf16, tag="alpha", bufs=1)
                        wstate = {}

                        def s1a(w):
                            c0, ncwW = coff[w], ncw[w]
                            ne = ncwW * 128
                            As = ephp.tile([128, ncmax, 64], f32, tag="as", bufs=2)
                            nc.gpsimd.dma_gather(
                                As[:, :ncwW, :], al_f32,
                                idxs_t[:, c0:c0 + ncwW, :], ne, ne, 64, elem_step=64,
                                queue_num=(2 * w) % 4)
                            maskw = mwp.tile([128, ncmax, 128], bf16, tag="mw", bufs=2)
                            nc.scalar.dma_start(
                                maskw[:, :ncwW, :],
                                mask_d[:, c0 * 128:(c0 + ncwW) * 128].rearrange(
                                    "p (c d) -> p c d", c=ncwW))
                            maskTw = mwp.tile([128, ncmax, 128], bf16, tag="mwT", bufs=2)
                            nc.scalar.dma_start(
                                maskTw[:, :ncwW, :],
                                maskT_d[:, c0 * 128:(c0 + ncwW) * 128].rearrange(
                                    "p (c d) -> p c d", c=ncwW))
                            # ald of the window's own dst nodes, scattered to edge
                            # slots via the one-hot maskT (replaces a dma_gather)
                            ald_f = ephp.tile([128, 8], f32, tag="aldf", bufs=3)
                            nc.sync.dma_start(
                                ald_f[:, :H], ashard[w * 128:(w + 1) * 128, H:2 * H])
                            ald_b = ephp.tile([128, 8], bf16, tag="aldb", bufs=3)
                            nc.vector.tensor_copy(ald_b[:, :H], ald_f[:, :H])
                            wstate[w] = (As, maskw, maskTw, ald_b)

                        def s1b(w):
                            c0, ncwW = coff[w], ncw[w]
                            As, maskw, maskTw, ald_b = wstate[w]
                            ps_ad = mmp.tile([128, ncmax, 8], f32, tag="smAd", bufs=2)
                            for c in range(ncwW):
                                nc.tensor.matmul(
                                    ps_ad[:, c, :H], maskTw[:, c, :], ald_b[:, :H],
                                    start=True, stop=True)
                            ev = ephp.tile([128, ncmax, 8], f32, tag="ev", bufs=2)
                            nc.vector.tensor_tensor(
                                ev[:, :ncwW, :H], As[:, :ncwW, 0:H], ps_ad[:, :ncwW, :H], ALU.add)
                            nc.vector.scalar_tensor_tensor(
                                ev[:, :ncwW, :H], ev[:, :ncwW, :H], 0.2, ev[:, :ncwW, :H],
                                ALU.mult, ALU.max)
                            nc.vector.tensor_scalar_min(ev[:, :ncwW, :H], ev[:, :ncwW, :H], EXP_CLAMP)
                            wv = ephp.tile([128, ncmax, 8], bf16, tag="wv", bufs=2)
                            nc.scalar.activation(wv[:, :ncwW, :H], ev[:, :ncwW, :H], ACTF.Exp)
                            wstate[w] = (wv, maskw, maskTw)

                        def s2(w):
                            c0, ncwW = coff[w], ncw[w]
                            wv, maskw, maskTw = wstate[w]
                            ps_den = mmp.tile([128, 8], f32, tag="smA", bufs=2)
                            for c in range(ncwW):
                                nc.tensor.matmul(
                                    ps_den[:, :H], maskw[:, c, :], wv[:, c, :H],
                                    start=(c == 0), stop=(c == ncwW - 1))
                            rden_f = ephp.tile([128, 8], f32, tag="rdenf", bufs=2)
                            nc.vector.tensor_scalar_max(rden_f[:, :H], ps_den[:, :H], DEN_TINY)
                            rden2 = ephp.tile([128, 8], f32, tag="rden2", bufs=2)
                            nc.vector.reciprocal(rden2[:, :H], rden_f[:, :H])
                            rden = ephp.tile([128, 8], bf16, tag="rden", bufs=2)
                            nc.vector.tensor_copy(rden[:, :H], rden2[:, :H])
                            wstate[w] = (wv, maskw, maskTw, rden)

                        def s3(w):
                            c0, ncwW = coff[w], ncw[w]
                            wv, maskw, maskTw, rden = wstate.pop(w)
                            # batched: all chunk matmuls into one PSUM tile, then a
                            # single vector mult -- no PE<->vector ping-pong stalls
                            ps_exp = mmp.tile([128, ncmax, 8], f32, tag="smAd", bufs=2)
                            for c in range(ncwW):
                                nc.tensor.matmul(
                                    ps_exp[:, c, :H], maskTw[:, c, :], rden[:, :H],
                                    start=True, stop=True)
                            nc.vector.tensor_tensor(
                                alpha_all[:, c0:c0 + ncwW, :H], wv[:, :ncwW, :H],
                                ps_exp[:, :ncwW, :H], ALU.mult)

                        qa = [(s1a, w) for w in range(NW)]
                        qb = [(f, w) for w in range(NW) for f in (s1b, s2, s3)]
                        na = nb = 0

                        def pop_stage():
                            nonlocal na, nb
                            if qa and na < nb // 3 + 3:
                                f, w = qa.pop(0); na += 1
                            elif qb and (nb // 3 + 2 <= na or not qa):
                                f, w = qb.pop(0); nb += 1
                            elif qa:
                                f, w = qa.pop(0); na += 1
                            else:
                                return False
                            f(w)
                            return True

                        per_slot = 2 if li == 3 else 1
                        slot = 0
                        for fci, (fo, fw) in enumerate(fcs):
                            kgrps = [(k0, min(7, kch - k0)) for k0 in range(0, kch, 7)]
                            rts = []
                            for (k0, kn) in kgrps:
                                rt = rhsp.tile([128, 7, 512], bf16, tag="rhs")
                                nc.sync.dma_start(
                                    rt[:, :kn, :fw],
                                    rhs_d[li][k0 * 128:(k0 + kn) * 128, fo:fo + fw]
                                    .rearrange("(k p) n -> p k n", p=128))
                                rts.append(rt)
                            for t in range(NT):
                                # deferred biasT chunks first: the eviction below
                                # reads biasT, so its writers must precede it
                                if fci >= 1:
                                    for _ in range(2):
                                        if pending_biasT:
                                            pending_biasT.pop(0)(mmp)
                                ps = mmp.tile([128, 512], f32, tag="mm")
                                for kc in range(kch):
                                    nc.tensor.matmul(
                                        ps[:, :fw],
                                        yT[:, kc, t * 128:(t + 1) * 128],
                                        rts[kc // 7][:, kc % 7, :fw],
                                        start=(kc == 0),
                                        stop=(kc == kch - 1))
                                hw = max(0, min(fw, fout - fo))
                                if hw > 0:
                                    # h' = rinv*(z.lnw @ W) + (lnb - mu*rinv*lnw)@W : the
                                    # table rows carry the exact LN'd h (see prep_params)
                                    st = stagep.tile([128, 512], bf16, tag="stg")
                                    nc.vector.scalar_tensor_tensor(
                                        st[:, :hw], ps[:, :hw], br[:, 1:2],
                                        biasT[:, fo:fo + hw], ALU.mult, ALU.subtract)
                                    if fo < half:
                                        nc.sync.dma_start(
                                            shard_a[t * 128:(t + 1) * 128, fo:fo + hw], st[:, :hw])
                                    else:
                                        nc.sync.dma_start(
                                            shard_b[t * 128:(t + 1) * 128, fo - half:fo - half + hw],
                                            st[:, :hw])
                                if hw < fw:
                                    a0 = fo + hw - fout
                                    sa = stagep.tile([128, 16], f32, tag="stga")
                                    nc.vector.scalar_tensor_tensor(
                                        sa[:, :fw - hw], ps[:, hw:fw], br[:, 1:2],
                                        biasT[:, fout + a0:fout + a0 + fw - hw],
                                        ALU.mult, ALU.subtract)
                                    nc.sync.dma_start(
                                        ashard[t * 128:(t + 1) * 128, a0:a0 + fw - hw],
                                        sa[:, :fw - hw])
                                if fci >= 1:
                                    if slot >= 2:
                                        for _ in range(per_slot):
                                            pop_stage()
                                    slot += 1
                            if fci == 0:
                                nc.gpsimd.collective_compute(
                                    "AllGather", ALU.bypass, ins=[ashard[:]], outs=[aglob[:]],
                                    replica_groups=RG)
                            if fci == fci_a_done:
                                # drain s1a gathers first: they must hit the gpsimd
                                # queue before the collective blocks it
                                while qa:
                                    pop_stage()
                                nc.gpsimd.collective_compute(
                                    "AllGather", ALU.bypass, ins=[shard_a[:]], outs=[glob_a[:]],
                                    replica_groups=RG)
                        while qa or qb:
                            pop_stage()

                    nc.gpsimd.collective_compute(
                        "AllGather", ALU.bypass, ins=[shard_b[:]], outs=[glob_b[:]],
                        replica_groups=RG)
                else:
                    alpha_all = alpha1_t

                # ===== edge phase: for li>0 two passes over column halves so
                # the glob_b AllGather hides under pass A's gather+scatter work
                edgeps = tc.tile_pool(name=f"psB{li}", bufs=1, space="PSUM")
                edgep = edgeps.__enter__()
                if li < 3:
                    zT = ytp.tile([128, 28, NPAD], bf16, tag="yt")
                    stats = miscp.tile([128, 4 * NW], f32, tag="stats")
                    if li == 0:
                        nc.vector.memset(stats[:], 0.0)

                if li == 0 or li == 3:
                    passes = [(0, fout, None)]
                else:
                    passes = [(0, half, glob_a), (half, fout, glob_b)]
                pending_tr = []          # deferred transposes of the previous window
                for pi, (lo, hi, glob) in enumerate(passes):
                    for w in range(NW):
                        c0, ncwW = coff[w], ncw[w]
                        maskw = mwp.tile([128, ncmax, 128], bf16, tag="mw", bufs=2)
                        nc.scalar.dma_start(
                            maskw[:, :ncwW, :],
                            mask_d[:, c0 * 128:(c0 + ncwW) * 128].rearrange(
                                "p (c d) -> p c d", c=ncwW))

                        if li == 0:
                            ps_agg = edgep.tile([128, 2, 8, 128], f32, tag="out")
                        elif li == 3:
                            ps_out = edgep.tile([128, fout], f32, tag="out", bufs=2)
                        else:
                            ps_out = edgep.tile([128, half], f32, tag="out", bufs=2)

                        for cp in range(0, ncwW, 2):
                            cw = min(2, ncwW - cp)
                            # flat [128, cw*gw] gather tiles: one shared tag for all
                            # layers/widths (a flat slice stays contiguous)
                            if li == 0:
                                gw = 256
                                G = gathp.tile([128, 3072], bf16, tag="G")
                                nc.gpsimd.dma_gather(
                                    G[:].rearrange("p (c g) -> p c g", g=256)[:, :cw, :],
                                    xtab_d[:][:, 0:256],
                                    idxs_t[:, c0 + cp:c0 + cp + cw, :], cw * 128, cw * 128,
                                    256, elem_step=384, queue_num=(cp // 2) % 4)
                            elif li == 3:
                                gw = half
                                G = gathp.tile([128, 3072], bf16, tag="G")
                                nc.gpsimd.dma_gather(
                                    G[:].rearrange("p (c g) -> p c g", g=half)[:, :cw, :], glob_a[:],
                                    idxs_t[:, c0 + cp:c0 + cp + cw, :], cw * 128, cw * 128,
                                    half, elem_step=half, queue_num=(cp // 2) % 4)
                                Gb = gathp.tile([128, 1024], bf16, tag="Gb")
                                nc.gpsimd.dma_gather(
                                    Gb[:].rearrange("p (c g) -> p c g", g=tcol - half)[:, :cw, :], glob_b[:],
                                    idxs_t[:, c0 + cp:c0 + cp + cw, :], cw * 128, cw * 128,
                                    tcol - half, elem_step=tcol - half,
                                    queue_num=(cp // 2 + 2) % 4)
                            else:
                                gw = half if pi == 0 else tcol - half  # table row width
                                G = gathp.tile([128, 3072], bf16, tag="G")
                                nc.gpsimd.dma_gather(
                                    G[:].rearrange("p (c g) -> p c g", g=gw)[:, :cw, :], glob[:],
                                    idxs_t[:, c0 + cp:c0 + cp + cw, :], cw * 128, cw * 128,
                                    gw, elem_step=gw, queue_num=(cp // 2) % 4)
                            # batched alpha-weighted one-hot lhs: one vector op per pair
                            lhs = ephp.tile([128, 2, 8, 128], bf16, tag="lhs")
                            nc.vector.tensor_tensor(
                                lhs[:, :cw, :H, :],
                                maskw[:, cp:cp + cw, :].unsqueeze(2).broadcast_to(
                                    [128, cw, H, 128]),
                                alpha_all[:, c0 + cp:c0 + cp + cw, :H].unsqueeze(3).broadcast_to(
                                    [128, cw, H, 128]),
                                ALU.mult)
                            for c in range(cp, cp + cw):
                                gof = (c - cp) * gw
                                if li == 0:
                                    for kc in range(2):
                                        for hg in range(2):
                                            nc.tensor.matmul(
                                                ps_agg[:, kc, hg * 4:(hg + 1) * 4, :],
                                                G[:, gof + kc * 128:gof + (kc + 1) * 128],
                                                lhs[:, c - cp, hg * 4:(hg + 1) * 4, :],
                                                start=(c == 0), stop=False)
                                else:
                                    o = lo
                                    while o < hi:
                                        h = o // C
                                        e = min((h + 1) * C, (o // 512 + 1) * 512, hi)
                                        if li == 3 and o >= half:
                                            bof = (c - cp) * (tcol - half)
                                            rhs_g = Gb[:, bof + o - half:bof + e - half]
                                        else:
                                            rhs_g = G[:, gof + o - lo:gof + e - lo]
                                        nc.tensor.matmul(
                                            ps_out[:, o - lo:e - lo], lhs[:, c - cp, h, :],
                                            rhs_g,
                                            start=(c == 0 and o % 512 == 0), stop=False)
                                        o = e

                        if li == 0:
                            # xaggT came out of the scatter matmuls already transposed
                            xs = zp.tile([128, 2, 8, 128], bf16, tag="z", bufs=3)
                            nc.vector.tensor_copy(xs[:], ps_agg[:])
                            ps_out = edgep.tile([128, fout], f32, tag="out")
                            for h in range(H):
                                o = h * C
                                while o < (h + 1) * C:
                                    e = min((o // 512 + 1) * 512, (h + 1) * C)
                                    for kc in range(2):
                                        nc.tensor.matmul(
                                            ps_out[:, o:e], xs[:, kc, h, :],
                                            rt1[:, kc, o:e],
                                            start=(kc == 0 and o % 512 == 0), stop=False)
                                    o = e
                            lo2, hi2 = 0, fout
                        else:
                            lo2, hi2 = lo, hi
                        # bias add
                        o = lo2
                        while o < hi2:
                            e = min(o + 512, hi2)
                            nc.tensor.matmul(
                                ps_out[:, o - lo2:e - lo2], ones_t[:, w, :], brow[:, o:e],
                                start=False, stop=(e == hi2))
                            o = e

                        if li < 3:
                            if li == 0:
                                subr = [(0, 1792, 0), (1792, 3584, 1)]
                            else:
                                subr = [(lo2, hi2, pi)]
                            # flush the previous window's transposes now: its z is
                            # long since evicted, so the PE never waits on Scalar
                            for tr in pending_tr:
                                tr()
                            pending_tr = []
                            for (sl, sh, spi) in subr:
                                z = zp.tile([128, 1792], bf16, tag="z", bufs=3)
                                nc.scalar.activation(
                                    z[:, :sh - sl], ps_out[:, sl - lo2:sh - lo2], ACTF.Relu,
                                    accum_out=stats[:, spi * NW + w:spi * NW + w + 1])
                                sq = zp.tile([128, 1792], bf16, tag="z", bufs=3)
                                nc.scalar.activation(
                                    sq[:, :sh - sl], z[:, :sh - sl], ACTF.Square,
                                    accum_out=stats[:, 2 * NW + spi * NW + w:2 * NW + spi * NW + w + 1])

                                def make_tr(z=z, sl=sl, sh=sh, w=w):
                                    def tr():
                                        for q in range(sl // 128, sh // 128, 4):
                                            qn = min(4, sh // 128 - q)
                                            ps_t = edgep.tile([128, 4, 128], bf16, tag="sm")
                                            for j in range(qn):
                                                nc.tensor.matmul(
                                                    ps_t[:, j, :],
                                                    z[:, (q + j - sl // 128) * 128:(q + j - sl // 128 + 1) * 128],
                                                    ident_t[:], is_transpose=True,
                                                    start=(j == 0), stop=(j == qn - 1))
                                            nc.scalar.activation(
                                                zT[:, q:q + qn, w * 128:(w + 1) * 128],
                                                ps_t[:, :qn, :], ACTF.Copy)
                                    return tr
                                pending_tr.append(make_tr())
                        else:
                            zf = zp.tile([128, 1024], f32, tag="z", bufs=3)
                            nc.vector.tensor_copy(zf[:, :hi2 - lo2], ps_out[:, :hi2 - lo2])
                            rows = min(128, NPC - w * 128)
                            nc.sync.dma_start(
                                out_d[w * 128:w * 128 + rows, lo2:hi2], zf[:rows, :hi2 - lo2])
                for tr in pending_tr:
                    tr()
                pending_tr = []

                # ===== graph LayerNorm + next yT
                if li < 3:
                    sdram = dram.tile([1, 64], f32, tag=f"sd{li}")
                    sglob = dram.tile([1, 64], f32, addr_space="Shared", tag=f"sg{li}")
                    ps_s = edgep.tile([1, 4 * NW], f32, tag="sm")
                    nc.tensor.matmul(ps_s[:], ones128[:], stats[:], start=True, stop=True)
                    ssum = miscp.tile([1, 4], f32, tag="ssum")
                    nc.vector.tensor_reduce(ssum[:, 0:1], ps_s[:, 0:2 * NW], AX.X, ALU.add)
                    nc.vector.tensor_reduce(ssum[:, 1:2], ps_s[:, 2 * NW:4 * NW], AX.X, ALU.add)
                    nc.sync.dma_start(sdram[:, 0:2], ssum[:, 0:2])
                    nc.gpsimd.collective_compute(
                        "AllReduce", ALU.add, ins=[sdram[:]], outs=[sglob[:]],
                        replica_groups=RG)
                    gsum = miscp.tile([1, 8], f32, tag="gsum")
                    nc.sync.dma_start(gsum[:, 0:2], sglob[:, 0:2])
                    sc = miscp.tile([1, 8], f32, tag="sc")
                    inv_cnt = 1.0 / (float(N_NODES) * fout)
                    nc.vector.tensor_scalar_mul(sc[:, 0:2], gsum[:, 0:2], inv_cnt)
                    nc.vector.tensor_tensor(sc[:, 2:3], sc[:, 0:1], sc[:, 0:1], ALU.mult)
                    nc.vector.tensor_sub(sc[:, 3:4], sc[:, 1:2], sc[:, 2:3])
                    nc.vector.tensor_scalar_add(sc[:, 3:4], sc[:, 3:4], 1e-5)
                    nc.scalar.sqrt(sc[:, 4:5], sc[:, 3:4])
                    nc.vector.reciprocal(sc[:, 5:6], sc[:, 4:5])
                    mr = miscp.tile([1, 2], f32, tag="mr")
                    nc.vector.tensor_copy(mr[:, 0:1], sc[:, 0:1])
                    nc.vector.tensor_copy(mr[:, 1:2], sc[:, 5:6])
                    ps_b = edgep.tile([128, 2], f32, tag="sm")
                    nc.tensor.matmul(ps_b[:], onesT[:], mr[:], start=True, stop=True)
                    br = miscp.tile([128, 2], f32, tag=f"br{li}")
                    nc.vector.tensor_copy(br[:], ps_b[:])
                    c128 = miscp.tile([128, 1], f32, tag=f"c{li}")
                    nc.vector.tensor_tensor(c128[:], br[:, 0:1], br[:, 1:2], ALU.mult)
                    # next dense's rinv-independent row consts, negated:
                    # biasT = (mu*rinv)*r2 - r1  (evictions subtract it).
                    # Only the al-columns chunk is built here (the next dense's
                    # fci=0 evictions read it immediately); the rest is deferred
                    # into the next dense loop, hidden under its matmuls.
                    wnx = FOUT_L[li + 1] + 2 * HEADS_L[li + 1]
                    wnx0 = (FOUT_L[li + 1] // 256) * 256
                    biasT = miscp.tile([128, 3088], f32, tag="biasT", bufs=1)

                    def bt_chunk(o, e, pool, tag, pbufs=1, c128=c128, li=li, biasT=biasT):
                        r2row = miscp.tile([1, 256], f32, tag="r2row", bufs=2)
                        nc.sync.dma_start(r2row[:, :e - o], r2_d[li + 1][:, o:e])
                        r1row = miscp.tile([1, 256], f32, tag="r1row", bufs=2)
                        nc.sync.dma_start(r1row[:, :e - o], r1_d[li + 1][:, o:e])
                        rrow = miscp.tile([1, 256], f32, tag="rrow", bufs=2)
                        nc.vector.scalar_tensor_tensor(
                            rrow[:, :e - o], r2row[:, :e - o], c128[0:1, :],
                            r1row[:, :e - o], ALU.mult, ALU.subtract)
                        ps_bt = pool.tile([128, 256], f32, tag=tag, bufs=pbufs)
                        nc.tensor.matmul(
                            ps_bt[:, :e - o], onesT[:], rrow[:, :e - o],
                            start=True, stop=True)
                        nc.vector.tensor_copy(biasT[:, o:e], ps_bt[:, :e - o])

                    bt_chunk(wnx0, wnx, edgep, "sm")
                    o = 0
                    while o < wnx0:
                        e = min(o + 256, wnx0)
                        pending_biasT.append(
                            lambda pool, o=o, e=e: bt_chunk(o, e, pool, "smA", pbufs=2))
                        o = e
                    yT = zT
                    if debug:
                        nc.sync.dma_start(
                            dbg_zt[li][:], zT[:].rearrange("p q n -> p (q n)"))
                        nc.sync.dma_start(dbg_st[li][:], sc[:])
                edgeps.__exit__(None, None, None)

    nc.compile()
    _CACHE[key] = nc
    return nc


# ---------------------------------------------------------------- entry point
def make_in_maps(inputs):
    ncw, per_core = prep_edges(inputs["edge_index"])
    params = prep_params(inputs)
    alpha1 = prep_alpha1(per_core, ncw, params.pop("_al1"))
    in_maps = []
    for k in range(N_CORES):
        m = dict(params)
        m.update({kk: vv for kk, vv in per_core[k].items() if not kk.startswith("_")})
        m["alpha1"] = alpha1[k]
        in_maps.append(m)
    return ncw, in_maps


def kernel(**inputs):
    _install_ntff_hook()
    from concourse.bass_utils import run_bass_kernel_spmd

    ncw, in_maps = make_in_maps(inputs)
    nc = build(ncw)
    res = run_bass_kernel_spmd(nc, in_maps, core_ids=list(range(N_CORES)), trace=False)
    out = np.concatenate([res.results[k]["out"] for k in range(N_CORES)], axis=0)
    return out.astype(np.float32)



# revision 26
# speedup vs baseline: 1.1000x; 1.1000x over previous
"""GATNet (4-layer GAT, 10000 nodes / 50000 edges + self-loops) on 8 Trainium2 NeuronCores.

Self-contained: builds per-core shards on the host (edge bucketing by destination,
one-hot scatter masks, gather index tables), compiles one SPMD Bass program, runs it
on cores 0-7 via run_bass_kernel_spmd, and reassembles the full [10000, 1000] output.

Structure per layer:
  dense h = y @ W^T (bf16, attention projections folded as extra rhs columns).
  The previous layer's graph-LN is folded in: diag(lnw) is baked into the rhs
  weights on the host; the runtime rinv scalar and the (lnb - mu*rinv*lnw)@W
  row const (broadcast via a K=1 matmul) are applied at the PSUM->SBUF
  eviction on the Vector engine, so the h table holds the exact LN'd h and
  the dense consumes raw (pre-LN) zT -- no LN apply pass, no stats on the
  dense critical path.
  -> small AllGather of the per-node attention scores (al); the softmax
     pre-phase (als gathers; ald via one-hot maskT matmuls from the local al
     shard; e-values, segment denominators, reciprocals, alphas) overlaps the
     dense; the two big half-table AllGathers are emitted after it (collective
     instructions block the gpsimd queue until completion, so the pre-phase
     gathers must be issued first)
  -> gather phase: dma_gather of source h rows; one-hot scatter matmuls with
     the per-edge alpha folded into the stationary operand; bias via K=1
     matmul; ReLU eviction (ACT, fused row sums) with all PSUM->SBUF copies on
     the Vector engine; PE-transpose into feature-major for the next lhsT
  -> graph-LN stats via tiny AllReduce -> (mu, rinv) broadcast + next biasT.
Layer 1 never materializes h: by linearity sum_e alpha_e * (x W)[src_e] =
(sum_e alpha_e x[src_e]) W, so it scatters raw x rows (256 wide) and applies W1
once per destination window; its attention scores are computed exactly on the host.
"""
import sys
import types

import numpy as np
import ml_dtypes

BF16 = ml_dtypes.bfloat16

N_NODES = 10000
N_CORES = 8
NPC = 1250
NPAD = 1280
NT = 10
NW = 10
HEADS_L = [8, 8, 8, 1]
C_L = [448, 384, 256, 1000]
FIN_L = [256, 3584, 3072, 2048]
FOUT_L = [3584, 3072, 2048, 1000]
TCOL_L = [256, 3072, 2048, 1024]    # bf16 columns of the gather table (L1: raw x)
EXP_CLAMP = 35.0
DEN_TINY = 1e-30


def _install_ntff_hook():
    if "antenv.axon_hooks" in sys.modules:
        return
    try:
        import antenv
        from trn_agent_boot.trn_boot import _ntff_profile_via_ctypes
    except ImportError:
        return
    mod = types.ModuleType("antenv.axon_hooks")
    state = {"hook": None}
    mod.set_axon_ntff_profile_hook = lambda h: state.__setitem__("hook", h)
    mod.get_axon_ntff_profile_hook = lambda: state["hook"]
    sys.modules["antenv.axon_hooks"] = mod
    antenv.axon_hooks = mod
    mod.set_axon_ntff_profile_hook(_ntff_profile_via_ctypes("/opt/axon/libaxon_pjrt.so"))


# ---------------------------------------------------------------- host prep
def _table_row(n):
    return NPAD * (n // NPC) + (n % NPC)


def _wrap16(idx_chunk):
    w = idx_chunk.reshape(8, 16).T
    return np.tile(w, (8, 1)).astype(np.int16)


def prep_edges(edge_index):
    src = np.asarray(edge_index[0], dtype=np.int64)
    dst = np.asarray(edge_index[1], dtype=np.int64)
    src = np.concatenate([src, np.arange(N_NODES, dtype=np.int64)])
    dst = np.concatenate([dst, np.arange(N_NODES, dtype=np.int64)])

    buckets = [[[] for _ in range(NW)] for _ in range(N_CORES)]
    core_of = dst // NPC
    win_of = (dst % NPC) // 128
    order = np.argsort(dst, kind="stable")
    for e in order:
        buckets[core_of[e]][win_of[e]].append(e)

    ncw = []
    for w in range(NW):
        mx = max(len(buckets[k][w]) for k in range(N_CORES))
        ncw.append(max(1, -(-mx // 128)))
    nch = sum(ncw)

    per_core = []
    for k in range(N_CORES):
        idx_s = np.zeros((128, nch * 8), np.int16)
        idx_d = np.zeros((128, nch * 8), np.int16)
        mask = np.zeros((128, nch, 128), np.float32)
        maskT = np.zeros((128, nch, 128), np.float32)
        esrc = np.zeros((nch, 128), np.int64)
        edst = np.zeros((nch, 128), np.int64)
        ereal = np.zeros((nch, 128), bool)
        c0 = 0
        for w in range(NW):
            edges = buckets[k][w]
            for c in range(ncw[w]):
                part = edges[c * 128:(c + 1) * 128]
                srows = np.zeros(128, np.int64)
                drows = np.zeros(128, np.int64)
                for i, e in enumerate(part):
                    srows[i] = _table_row(src[e])
                    drows[i] = _table_row(dst[e])
                    esrc[c0 + c, i] = src[e]
                    edst[c0 + c, i] = dst[e]
                    ereal[c0 + c, i] = True
                    d_local = (dst[e] % NPC) - 128 * w
                    mask[i, c0 + c, d_local] = 1.0
                    maskT[d_local, c0 + c, i] = 1.0
                idx_s[:, (c0 + c) * 8:(c0 + c + 1) * 8] = _wrap16(srows)
                idx_d[:, (c0 + c) * 8:(c0 + c + 1) * 8] = _wrap16(drows)
            c0 += ncw[w]
        per_core.append(dict(
            idxs=idx_s,
            mask=mask.reshape(128, nch * 128).astype(BF16),
            maskT=maskT.reshape(128, nch * 128).astype(BF16),
            _esrc=esrc, _edst=edst, _ereal=ereal,
        ))
    return tuple(ncw), per_core


def prep_alpha1(per_core, ncw, al1):
    """Exact layer-1 softmax on the host: alpha[e, h] per (chunk, slot)."""
    nch = sum(ncw)
    als = al1[:, :8].astype(np.float64)
    ald = al1[:, 8:].astype(np.float64)
    out = []
    for pc in per_core:
        esrc, edst, ereal = pc["_esrc"], pc["_edst"], pc["_ereal"]
        e = als[esrc] + ald[edst]                      # [nch, 128, 8]
        e = np.maximum(e, 0.2 * e)
        wv = np.exp(np.minimum(e, EXP_CLAMP)) * ereal[:, :, None]
        den = np.zeros((NPC, 8))
        np.add.at(den, (edst % NPC).reshape(-1), wv.reshape(-1, 8))
        alpha = wv / np.maximum(den[(edst % NPC)], 1e-300)
        # device layout: [part=slot, nch*8]
        return_arr = np.ascontiguousarray(
            alpha.transpose(1, 0, 2).reshape(128, nch * 8)).astype(BF16)
        out.append(return_arr)
    return out


def prep_params(inputs):
    p = {}
    x64 = np.asarray(inputs["x"], np.float64)
    al1 = None
    for li in range(4):
        H, C, fin, fout = HEADS_L[li], C_L[li], FIN_L[li], FOUT_L[li]
        W = np.asarray(inputs[f"W{li+1}"], np.float64)
        a_src = np.asarray(inputs[f"a_src{li+1}"], np.float64)
        a_dst = np.asarray(inputs[f"a_dst{li+1}"], np.float64)
        a_blk_s = np.zeros((fout, H), np.float64)
        a_blk_d = np.zeros((fout, H), np.float64)
        for h in range(H):
            a_blk_s[h * C:(h + 1) * C, h] = a_src[h]
            a_blk_d[h * C:(h + 1) * C, h] = a_dst[h]
        rhs = np.concatenate([W.T, W.T @ a_blk_s, W.T @ a_blk_d], axis=1)
        # graph-LN of the previous layer folded in: y = rinv*(z*lnw) + (lnb - mu*rinv*lnw)
        # so h = y@rhs = rinv*(z @ diag(lnw)@rhs) + r1 - (mu*rinv)*r2 with the
        # static diag(lnw) baked into the device rhs and r1/r2 host rows.
        if li > 0:
            lwp = np.asarray(inputs[f"ln{li}_w"], np.float64)
            lbp = np.asarray(inputs[f"ln{li}_b"], np.float64)
            p[f"rhs{li+1}"] = np.ascontiguousarray(lwp[:, None] * rhs).astype(BF16)
            p[f"r1_{li+1}"] = (lbp @ rhs).reshape(1, fout + 2 * H).astype(np.float32)
            p[f"r2_{li+1}"] = (lwp @ rhs).reshape(1, fout + 2 * H).astype(np.float32)
        else:
            p[f"rhs{li+1}"] = np.ascontiguousarray(rhs).astype(BF16)
        p[f"brow{li+1}"] = np.asarray(inputs[f"b{li+1}"], np.float32).reshape(1, fout).astype(BF16)
        if li == 0:
            al1 = np.concatenate(
                [x64 @ (W.T @ a_blk_s), x64 @ (W.T @ a_blk_d)], axis=1).astype(np.float32)
    # packed x table, replicated: [10240 rows, 768 bytes] = 512B x bf16 + 256B f32 al
    xtab = np.zeros((N_CORES * NPAD, 768), np.uint8)
    rows = _table_row(np.arange(N_NODES))
    xb = np.asarray(inputs["x"], np.float32).astype(BF16)
    xtab[rows, :512] = xb.view(np.uint8)
    alpad = np.zeros((N_NODES, 64), np.float32)
    alpad[:, :16] = al1
    xtab[rows, 512:768] = alpad.view(np.uint8)
    p["xtab"] = xtab.view(BF16)
    p["_al1"] = al1
    p["ident"] = np.eye(128, dtype=BF16)
    ones_b = np.zeros((1, NW * 128), np.float32)
    ones_b[0, :NPC] = 1.0
    p["ones_b"] = ones_b.astype(BF16)
    return p


# ---------------------------------------------------------------- device build
_CACHE = {}


def build(ncw, debug=False):
    key = (tuple(ncw), debug)
    if key in _CACHE:
        return _CACHE[key]

    import concourse.bacc as bacc
    import concourse.mybir as mybir
    import concourse.tile as tile
    from concourse.library_config import mlp

    f32 = mybir.dt.float32
    bf16 = mybir.dt.bfloat16
    i16 = mybir.dt.int16
    AX = mybir.AxisListType
    ALU = mybir.AluOpType
    ACTF = mybir.ActivationFunctionType

    nch = sum(ncw)
    ncmax = max(ncw)
    coff = [0]
    for w in range(NW):
        coff.append(coff[-1] + ncw[w])
    nc = bacc.Bacc("TRN2", num_swdge_queues=4)

    xtab_d = nc.declare_dram_parameter("xtab", [N_CORES * NPAD, 384], bf16, isOutput=False)
    rhs_d, brow_d, r1_d, r2_d = [], [], [None], [None]
    for li in range(4):
        H, fout, fin = HEADS_L[li], FOUT_L[li], FIN_L[li]
        rhs_d.append(nc.declare_dram_parameter(f"rhs{li+1}", [fin, fout + 2 * H], bf16, isOutput=False))
        brow_d.append(nc.declare_dram_parameter(f"brow{li+1}", [1, fout], bf16, isOutput=False))
        if li > 0:
            r1_d.append(nc.declare_dram_parameter(f"r1_{li+1}", [1, fout + 2 * H], f32, isOutput=False))
            r2_d.append(nc.declare_dram_parameter(f"r2_{li+1}", [1, fout + 2 * H], f32, isOutput=False))
    alpha1_d = nc.declare_dram_parameter("alpha1", [128, nch * 8], bf16, isOutput=False)
    idxs_d = nc.declare_dram_parameter("idxs", [128, nch * 8], i16, isOutput=False)
    mask_d = nc.declare_dram_parameter("mask", [128, nch * 128], bf16, isOutput=False)
    maskT_d = nc.declare_dram_parameter("maskT", [128, nch * 128], bf16, isOutput=False)
    ones_d = nc.declare_dram_parameter("ones_b", [1, NW * 128], bf16, isOutput=False)
    ident_d = nc.declare_dram_parameter("ident", [128, 128], bf16, isOutput=False)
    out_d = nc.declare_dram_parameter("out", [NPC, 1000], f32, isOutput=True)
    dbg_zt, dbg_st = [], []
    if debug:
        for li in range(3):
            dbg_zt.append(nc.declare_dram_parameter(
                f"dbg_zt{li}", [128, 28 * NPAD], bf16, isOutput=True))
            dbg_st.append(nc.declare_dram_parameter(
                f"dbg_st{li}", [1, 8], f32, isOutput=True))

    RG = [list(range(N_CORES))]

    with tile.TileContext(nc) as tc:
        with (
            tc.tile_pool(name="const", bufs=1) as constp,
            tc.tile_pool(name="yt", bufs=1) as ytp,
            tc.tile_pool(name="rhs", bufs=5) as rhsp,
            tc.tile_pool(name="stage", bufs=2) as stagep,
            tc.tile_pool(name="gath", bufs=2) as gathp,
            tc.tile_pool(name="mw", bufs=2) as mwp,
            tc.tile_pool(name="eph", bufs=2) as ephp,
            tc.tile_pool(name="z", bufs=2) as zp,
            tc.tile_pool(name="misc", bufs=2) as miscp,
            tc.tile_pool(name="dram", bufs=1, space="DRAM") as dram,
        ):
            nc.gpsimd.load_library(mlp)

            idxs_t = constp.tile([128, nch, 8], i16, tag="idxs")
            nc.sync.dma_start(idxs_t[:], idxs_d[:].rearrange("p (c d) -> p c d", c=nch))
            ones_t = constp.tile([1, NW, 128], bf16, tag="onesb")
            nc.sync.dma_start(ones_t[:], ones_d[:].rearrange("p (w d) -> p w d", w=NW))
            ident_t = constp.tile([128, 128], bf16, tag="ident")
            nc.sync.dma_start(ident_t[:], ident_d[:])
            ones128 = constp.tile([128, 1], f32, tag="ones128")
            nc.vector.memset(ones128[:], 1.0)
            onesT = constp.tile([1, 128], f32, tag="onesT")
            nc.vector.memset(onesT[:], 1.0)
            alpha1_t = constp.tile([128, nch, 8], bf16, tag="alpha1")
            nc.sync.dma_start(alpha1_t[:], alpha1_d[:].rearrange("p (c d) -> p c d", c=nch))
            # resident W1 rhs (small; needed per destination window in layer 1)
            rt1 = constp.tile([128, 2, 3584], bf16, tag="rt1")
            nc.sync.dma_start(
                rt1[:], rhs_d[0][:, 0:3584].rearrange("(k p) n -> p k n", p=128))

            yT = None  # produced by each layer's LN for the next layer
            pending_biasT = []  # deferred biasT chunk builders for the next dense

            biasT = None  # [128, fout+2H] f32: rinv-independent LN row consts for this dense
            br = None     # [128, 2] f32: (mu, rinv) of previous layer's graph-LN

            for li in range(4):
                H, C, fin, fout = HEADS_L[li], C_L[li], FIN_L[li], FOUT_L[li]
                tcol = TCOL_L[li]
                kch = fin // 128
                acols = 2 * H

                brow = constp.tile([1, 3584], bf16, tag="brow")
                nc.sync.dma_start(brow[:, :fout], brow_d[li][:])

                if li > 0:
                    # ===== dense: h = y @ W^T (+ al columns); al chunk FIRST so the
                    # small al AllGather + softmax pre-phase overlap the dense phase
                    half = [None, 2048, 1024, 512][li]
                    shard_a = dram.tile([NPAD, half], bf16, tag=f"sharda{li}")
                    shard_b = dram.tile([NPAD, tcol - half], bf16, tag=f"shardb{li}")
                    glob_a = dram.tile([N_CORES * NPAD, half], bf16, addr_space="Shared", tag=f"globa{li}")
                    glob_b = dram.tile([N_CORES * NPAD, tcol - half], bf16, addr_space="Shared", tag=f"globb{li}")
                    ashard = dram.tile([NPAD, 64], f32, tag=f"ashard{li}")
                    aglob = dram.tile([N_CORES * NPAD, 64], f32, addr_space="Shared", tag=f"aglob{li}")
                    fcs = [(fout, acols)]
                    o = 0
                    while o < fout:
                        w_ = min(512, fout - o)
                        fcs.append((o, w_))
                        o += w_
                    # fci index after which all shard_a columns are written
                    fci_a_done = 0
                    o = 0
                    for fci, (fo, fw) in enumerate(fcs):
                        if fci > 0 and fo + fw <= half:
                            fci_a_done = fci
                    with tc.tile_pool(name=f"psA{li}", bufs=4, space="PSUM") as mmp:
                        # ---- softmax pre-phase, emitted as pipelined stages
                        # interleaved between dense row-tiles so PE/vector/
                        # scalar work overlaps the dense instead of trailing it
                        al_f32 = aglob[:]
                        alpha_all = ephp.tile([128, nch, 8], bf16, tag="alpha", bufs=1)
                        wstate = {}

                        def s1a(w):
                            c0, ncwW = coff[w], ncw[w]
                            ne = ncwW * 128
                            As = ephp.tile([128, ncmax, 64], f32, tag="as", bufs=2)
                            nc.gpsimd.dma_gather(
                                As[:, :ncwW, :], al_f32,
                                idxs_t[:, c0:c0 + ncwW, :], ne, ne, 64, elem_step=64,
                                queue_num=(2 * w) % 4)
                            maskw = mwp.tile([128, ncmax, 128], bf16, tag="mw", bufs=2)
                            nc.scalar.dma_start(
                                maskw[:, :ncwW, :],
                                mask_d[:, c0 * 128:(c0 + ncwW) * 128].rearrange(
                                    "p (c d) -> p c d", c=ncwW))
                            maskTw = mwp.tile([128, ncmax, 128], bf16, tag="mwT", bufs=2)
                            nc.scalar.dma_start(
                                maskTw[:, :ncwW, :],
                                maskT_d[:, c0 * 128:(c0 + ncwW) * 128].rearrange(
                                    "p (c d) -> p c d", c=ncwW))
                            # ald of the window's own dst nodes, scattered to edge
                            # slots via the one-hot maskT (replaces a dma_gather)
                            ald_f = ephp.tile([128, 8], f32, tag="aldf", bufs=3)
                            nc.sync.dma_start(
                                ald_f[:, :H], ashard[w * 128:(w + 1) * 128, H:2 * H])
                            ald_b = ephp.tile([128, 8], bf16, tag="aldb", bufs=3)
                            nc.vector.tensor_copy(ald_b[:, :H], ald_f[:, :H])
                            wstate[w] = (As, maskw, maskTw, ald_b)

                        def s1b(w):
                            c0, ncwW = coff[w], ncw[w]
                            As, maskw, maskTw, ald_b = wstate[w]
                            ps_ad = mmp.tile([128, ncmax, 8], f32, tag="smAd", bufs=2)
                            for c in range(ncwW):
                                nc.tensor.matmul(
                                    ps_ad[:, c, :H], maskTw[:, c, :], ald_b[:, :H],
                                    start=True, stop=True)
                            ev = ephp.tile([128, ncmax, 8], f32, tag="ev", bufs=2)
                            nc.vector.tensor_tensor(
                                ev[:, :ncwW, :H], As[:, :ncwW, 0:H], ps_ad[:, :ncwW, :H], ALU.add)
                            nc.vector.scalar_tensor_tensor(
                                ev[:, :ncwW, :H], ev[:, :ncwW, :H], 0.2, ev[:, :ncwW, :H],
                                ALU.mult, ALU.max)
                            nc.vector.tensor_scalar_min(ev[:, :ncwW, :H], ev[:, :ncwW, :H], EXP_CLAMP)
                            wv = ephp.tile([128, ncmax, 8], bf16, tag="wv", bufs=2)
                            nc.scalar.activation(wv[:, :ncwW, :H], ev[:, :ncwW, :H], ACTF.Exp)
                            wstate[w] = (wv, maskw, maskTw)

                        def s2(w):
                            c0, ncwW = coff[w], ncw[w]
                            wv, maskw, maskTw = wstate[w]
                            ps_den = mmp.tile([128, 8], f32, tag="smA", bufs=2)
                            for c in range(ncwW):
                                nc.tensor.matmul(
                                    ps_den[:, :H], maskw[:, c, :], wv[:, c, :H],
                                    start=(c == 0), stop=(c == ncwW - 1))
                            rden_f = ephp.tile([128, 8], f32, tag="rdenf", bufs=2)
                            nc.vector.tensor_scalar_max(rden_f[:, :H], ps_den[:, :H], DEN_TINY)
                            rden2 = ephp.tile([128, 8], f32, tag="rden2", bufs=2)
                            nc.vector.reciprocal(rden2[:, :H], rden_f[:, :H])
                            rden = ephp.tile([128, 8], bf16, tag="rden", bufs=2)
                            nc.vector.tensor_copy(rden[:, :H], rden2[:, :H])
                            wstate[w] = (wv, maskw, maskTw, rden)

                        def s3(w):
                            c0, ncwW = coff[w], ncw[w]
                            wv, maskw, maskTw, rden = wstate.pop(w)
                            # batched: all chunk matmuls into one PSUM tile, then a
                            # single vector mult -- no PE<->vector ping-pong stalls
                            ps_exp = mmp.tile([128, ncmax, 8], f32, tag="smAd", bufs=2)
                            for c in range(ncwW):
                                nc.tensor.matmul(
                                    ps_exp[:, c, :H], maskTw[:, c, :], rden[:, :H],
                                    start=True, stop=True)
                            nc.vector.tensor_tensor(
                                alpha_all[:, c0:c0 + ncwW, :H], wv[:, :ncwW, :H],
                                ps_exp[:, :ncwW, :H], ALU.mult)

                        qa = [(s1a, w) for w in range(NW)]
                        qb = [(f, w) for w in range(NW) for f in (s1b, s2, s3)]
                        na = nb = 0

                        def pop_stage():
                            nonlocal na, nb
                            if qa and na < nb // 3 + 3:
                                f, w = qa.pop(0); na += 1
                            elif qb and (nb // 3 + 2 <= na or not qa):
                                f, w = qb.pop(0); nb += 1
                            elif qa:
                                f, w = qa.pop(0); na += 1
                            else:
                                return False
                            f(w)
                            return True

                        per_slot = 2 if li == 3 else 1
                        slot = 0
                        for fci, (fo, fw) in enumerate(fcs):
                            kgrps = [(k0, min(7, kch - k0)) for k0 in range(0, kch, 7)]
                            rts = []
                            for (k0, kn) in kgrps:
                                rt = rhsp.tile([128, 7, 512], bf16, tag="rhs")
                                nc.sync.dma_start(
                                    rt[:, :kn, :fw],
                                    rhs_d[li][k0 * 128:(k0 + kn) * 128, fo:fo + fw]
                                    .rearrange("(k p) n -> p k n", p=128))
                                rts.append(rt)
                            for t in range(NT):
                                # deferred biasT chunks first: the eviction below
                                # reads biasT, so its writers must precede it
                                if fci >= 1:
                                    for _ in range(2):
                                        if pending_biasT:
                                            pending_biasT.pop(0)(mmp)
                                ps = mmp.tile([128, 512], f32, tag="mm")
                                for kc in range(kch):
                                    nc.tensor.matmul(
                                        ps[:, :fw],
                                        yT[:, kc, t * 128:(t + 1) * 128],
                                        rts[kc // 7][:, kc % 7, :fw],
                                        start=(kc == 0),
                                        stop=(kc == kch - 1))
                                hw = max(0, min(fw, fout - fo))
                                if hw > 0:
                                    # h' = rinv*(z.lnw @ W) + (lnb - mu*rinv*lnw)@W : the
                                    # table rows carry the exact LN'd h (see prep_params)
                                    st = stagep.tile([128, 512], bf16, tag="stg")
                                    nc.vector.scalar_tensor_tensor(
                                        st[:, :hw], ps[:, :hw], br[:, 1:2],
                                        biasT[:, fo:fo + hw], ALU.mult, ALU.subtract)
                                    if fo < half:
                                        nc.sync.dma_start(
                                            shard_a[t * 128:(t + 1) * 128, fo:fo + hw], st[:, :hw])
                                    else:
                                        nc.sync.dma_start(
                                            shard_b[t * 128:(t + 1) * 128, fo - half:fo - half + hw],
                                            st[:, :hw])
                                if hw < fw:
                                    a0 = fo + hw - fout
                                    sa = stagep.tile([128, 16], f32, tag="stga")
                                    nc.vector.scalar_tensor_tensor(
                                        sa[:, :fw - hw], ps[:, hw:fw], br[:, 1:2],
                                        biasT[:, fout + a0:fout + a0 + fw - hw],
                                        ALU.mult, ALU.subtract)
                                    nc.sync.dma_start(
                                        ashard[t * 128:(t + 1) * 128, a0:a0 + fw - hw],
                                        sa[:, :fw - hw])
                                if fci >= 1:
                                    if slot >= 2:
                                        for _ in range(per_slot):
                                            pop_stage()
                                    slot += 1
                            if fci == 0:
                                nc.gpsimd.collective_compute(
                                    "AllGather", ALU.bypass, ins=[ashard[:]], outs=[aglob[:]],
                                    replica_groups=RG)
                            if fci == fci_a_done:
                                # drain s1a gathers first: they must hit the gpsimd
                                # queue before the collective blocks it
                                while qa:
                                    pop_stage()
                                nc.gpsimd.collective_compute(
                                    "AllGather", ALU.bypass, ins=[shard_a[:]], outs=[glob_a[:]],
                                    replica_groups=RG)
                        while qa or qb:
                            pop_stage()

                    nc.gpsimd.collective_compute(
                        "AllGather", ALU.bypass, ins=[shard_b[:]], outs=[glob_b[:]],
                        replica_groups=RG)
                else:
                    alpha_all = alpha1_t

                # ===== edge phase: for li>0 two passes over column halves so
                # the glob_b AllGather hides under pass A's gather+scatter work
                edgeps = tc.tile_pool(name=f"psB{li}", bufs=1, space="PSUM")
                edgep = edgeps.__enter__()
                if li < 3:
                    zT = ytp.tile([128, 28, NPAD], bf16, tag="yt")
                    stats = miscp.tile([128, 2 * NW], f32, tag="stats")

                # ---- gather + scatter phase (single pass; glob_a covers cols
                # [0:half] with half > tcol/2 so the trailing AG_b is small)
                for w in range(NW):
                    c0, ncwW = coff[w], ncw[w]
                    maskw = mwp.tile([128, ncmax, 128], bf16, tag="mw", bufs=2)
                    nc.scalar.dma_start(
                        maskw[:, :ncwW, :],
                        mask_d[:, c0 * 128:(c0 + ncwW) * 128].rearrange(
                            "p (c d) -> p c d", c=ncwW))

                    if li == 0:
                        ps_agg = edgep.tile([128, 2, 8, 128], f32, tag="out")
                    else:
                        ps_out = edgep.tile([128, fout], f32, tag="out")

                    for cp in range(0, ncwW, 2):
                        cw = min(2, ncwW - cp)
                        # flat gather tiles: one shared tag for all layers/widths
                        if li == 0:
                            G = gathp.tile([128, 4096], bf16, tag="G")
                            nc.gpsimd.dma_gather(
                                G[:].rearrange("p (c g) -> p c g", g=256)[:, :cw, :],
                                xtab_d[:][:, 0:256],
                                idxs_t[:, c0 + cp:c0 + cp + cw, :], cw * 128, cw * 128,
                                256, elem_step=384, queue_num=(cp // 2) % 4)
                        else:
                            G = gathp.tile([128, 4096], bf16, tag="G")
                            nc.gpsimd.dma_gather(
                                G[:].rearrange("p (c g) -> p c g", g=half)[:, :cw, :],
                                glob_a[:],
                                idxs_t[:, c0 + cp:c0 + cp + cw, :], cw * 128, cw * 128,
                                half, elem_step=half, queue_num=(cp // 2) % 4)
                            Gb = gathp.tile([128, 2048], bf16, tag="Gb")
                            nc.gpsimd.dma_gather(
                                Gb[:].rearrange("p (c g) -> p c g", g=tcol - half)[:, :cw, :],
                                glob_b[:],
                                idxs_t[:, c0 + cp:c0 + cp + cw, :], cw * 128, cw * 128,
                                tcol - half, elem_step=tcol - half,
                                queue_num=(cp // 2 + 2) % 4)
                        for c in range(cp, cp + cw):
                            lhs = ephp.tile([128, 8, 128], bf16, tag="lhs")
                            nc.vector.tensor_tensor(
                                lhs[:, :H, :],
                                maskw[:, c, :].unsqueeze(1).broadcast_to([128, H, 128]),
                                alpha_all[:, c0 + c, :H].unsqueeze(2).broadcast_to([128, H, 128]),
                                ALU.mult)
                            if li == 0:
                                gof = (c - cp) * 256
                                for kc in range(2):
                                    for hg in range(2):
                                        nc.tensor.matmul(
                                            ps_agg[:, kc, hg * 4:(hg + 1) * 4, :],
                                            G[:, gof + kc * 128:gof + (kc + 1) * 128],
                                            lhs[:, hg * 4:(hg + 1) * 4, :],
                                            start=(c == 0), stop=False)
                            else:
                                o = 0
                                while o < fout:
                                    h = o // C
                                    e = min((h + 1) * C, (o // 512 + 1) * 512, fout)
                                    if o < half:
                                        e = min(e, half)
                                        gof = (c - cp) * half
                                        rhs_g = G[:, gof + o:gof + e]
                                    else:
                                        bof = (c - cp) * (tcol - half)
                                        rhs_g = Gb[:, bof + o - half:bof + e - half]
                                    nc.tensor.matmul(
                                        ps_out[:, o:e], lhs[:, h, :], rhs_g,
                                        start=(c == 0 and o % 512 == 0), stop=False)
                                    o = e

                    if li == 0:
                        # xaggT came out of the scatter matmuls already transposed
                        xs = zp.tile([128, 2, 8, 128], bf16, tag="z")
                        nc.vector.tensor_copy(xs[:], ps_agg[:])
                        ps_out = edgep.tile([128, fout], f32, tag="out")
                        for h in range(H):
                            o = h * C
                            while o < (h + 1) * C:
                                e = min((o // 512 + 1) * 512, (h + 1) * C)
                                for kc in range(2):
                                    nc.tensor.matmul(
                                        ps_out[:, o:e], xs[:, kc, h, :],
                                        rt1[:, kc, o:e],
                                        start=(kc == 0 and o % 512 == 0), stop=False)
                                o = e
                    # bias add
                    o = 0
                    while o < fout:
                        e = min(o + 512, fout)
                        nc.tensor.matmul(
                            ps_out[:, o:e], ones_t[:, w, :], brow[:, o:e],
                            start=False, stop=(e == fout))
                        o = e

                    if li < 3:
                        z = zp.tile([128, 3584], bf16, tag="z")
                        nc.scalar.activation(
                            z[:, :fout], ps_out[:, :fout], ACTF.Relu,
                            accum_out=stats[:, w:w + 1])
                        sq = zp.tile([128, 3584], bf16, tag="z")
                        nc.vector.scalar_tensor_tensor(
                            sq[:, :fout], z[:, :fout], 1.0, z[:, :fout],
                            ALU.mult, ALU.mult,
                            accum_out=stats[:, NW + w:NW + w + 1])
                        for q in range(0, fout // 128, 4):
                            qn = min(4, fout // 128 - q)
                            ps_t = edgep.tile([128, 4, 128], bf16, tag="sm")
                            for j in range(qn):
                                nc.tensor.matmul(
                                    ps_t[:, j, :], z[:, (q + j) * 128:(q + j + 1) * 128],
                                    ident_t[:], is_transpose=True,
                                    start=(j == 0), stop=(j == qn - 1))
                            nc.vector.tensor_copy(
                                zT[:, q:q + qn, w * 128:(w + 1) * 128],
                                ps_t[:, :qn, :])
                    else:
                        zf = zp.tile([128, 1024], f32, tag="z")
                        nc.vector.tensor_copy(zf[:, :fout], ps_out[:, :fout])
                        rows = min(128, NPC - w * 128)
                        nc.sync.dma_start(out_d[w * 128:w * 128 + rows, :], zf[:rows, :fout])

                # ===== graph LayerNorm + next yT
                if li < 3:
                    sdram = dram.tile([1, 64], f32, tag=f"sd{li}")
                    sglob = dram.tile([1, 64], f32, addr_space="Shared", tag=f"sg{li}")
                    ps_s = edgep.tile([1, 2 * NW], f32, tag="sm")
                    nc.tensor.matmul(ps_s[:], ones128[:], stats[:], start=True, stop=True)
                    ssum = miscp.tile([1, 4], f32, tag="ssum")
                    nc.vector.tensor_reduce(ssum[:, 0:1], ps_s[:, 0:NW], AX.X, ALU.add)
                    nc.vector.tensor_reduce(ssum[:, 1:2], ps_s[:, NW:2 * NW], AX.X, ALU.add)
                    nc.sync.dma_start(sdram[:, 0:2], ssum[:, 0:2])
                    nc.gpsimd.collective_compute(
                        "AllReduce", ALU.add, ins=[sdram[:]], outs=[sglob[:]],
                        replica_groups=RG)
                    gsum = miscp.tile([1, 8], f32, tag="gsum")
                    nc.sync.dma_start(gsum[:, 0:2], sglob[:, 0:2])
                    sc = miscp.tile([1, 8], f32, tag="sc")
                    inv_cnt = 1.0 / (float(N_NODES) * fout)
                    nc.vector.tensor_scalar_mul(sc[:, 0:2], gsum[:, 0:2], inv_cnt)
                    nc.vector.tensor_tensor(sc[:, 2:3], sc[:, 0:1], sc[:, 0:1], ALU.mult)
                    nc.vector.tensor_sub(sc[:, 3:4], sc[:, 1:2], sc[:, 2:3])
                    nc.vector.tensor_scalar_add(sc[:, 3:4], sc[:, 3:4], 1e-5)
                    nc.scalar.sqrt(sc[:, 4:5], sc[:, 3:4])
                    nc.vector.reciprocal(sc[:, 5:6], sc[:, 4:5])
                    mr = miscp.tile([1, 2], f32, tag="mr")
                    nc.vector.tensor_copy(mr[:, 0:1], sc[:, 0:1])
                    nc.vector.tensor_copy(mr[:, 1:2], sc[:, 5:6])
                    ps_b = edgep.tile([128, 2], f32, tag="sm")
                    nc.tensor.matmul(ps_b[:], onesT[:], mr[:], start=True, stop=True)
                    br = miscp.tile([128, 2], f32, tag=f"br{li}")
                    nc.vector.tensor_copy(br[:], ps_b[:])
                    c128 = miscp.tile([128, 1], f32, tag=f"c{li}")
                    nc.vector.tensor_tensor(c128[:], br[:, 0:1], br[:, 1:2], ALU.mult)
                    # next dense's rinv-independent row consts, negated:
                    # biasT = (mu*rinv)*r2 - r1  (evictions subtract it).
                    # Only the al-columns chunk is built here (the next dense's
                    # fci=0 evictions read it immediately); the rest is deferred
                    # into the next dense loop, hidden under its matmuls.
                    wnx = FOUT_L[li + 1] + 2 * HEADS_L[li + 1]
                    wnx0 = (FOUT_L[li + 1] // 256) * 256
                    biasT = miscp.tile([128, 3088], f32, tag="biasT", bufs=1)

                    def bt_chunk(o, e, pool, tag, pbufs=1, c128=c128, li=li, biasT=biasT):
                        r2row = miscp.tile([1, 256], f32, tag="r2row", bufs=2)
                        nc.sync.dma_start(r2row[:, :e - o], r2_d[li + 1][:, o:e])
                        r1row = miscp.tile([1, 256], f32, tag="r1row", bufs=2)
                        nc.sync.dma_start(r1row[:, :e - o], r1_d[li + 1][:, o:e])
                        rrow = miscp.tile([1, 256], f32, tag="rrow", bufs=2)
                        nc.vector.scalar_tensor_tensor(
                            rrow[:, :e - o], r2row[:, :e - o], c128[0:1, :],
                            r1row[:, :e - o], ALU.mult, ALU.subtract)
                        ps_bt = pool.tile([128, 256], f32, tag=tag, bufs=pbufs)
                        nc.tensor.matmul(
                            ps_bt[:, :e - o], onesT[:], rrow[:, :e - o],
                            start=True, stop=True)
                        nc.vector.tensor_copy(biasT[:, o:e], ps_bt[:, :e - o])

                    bt_chunk(wnx0, wnx, edgep, "sm")
                    o = 0
                    while o < wnx0:
                        e = min(o + 256, wnx0)
                        pending_biasT.append(
                            lambda pool, o=o, e=e: bt_chunk(o, e, pool, "smA", pbufs=2))
                        o = e
                    yT = zT
                    if debug:
                        nc.sync.dma_start(
                            dbg_zt[li][:], zT[:].rearrange("p q n -> p (q n)"))
                        nc.sync.dma_start(dbg_st[li][:], sc[:])
                edgeps.__exit__(None, None, None)

    nc.compile()
    _CACHE[key] = nc
    return nc


# ---------------------------------------------------------------- entry point
def make_in_maps(inputs):
    ncw, per_core = prep_edges(inputs["edge_index"])
    params = prep_params(inputs)
    alpha1 = prep_alpha1(per_core, ncw, params.pop("_al1"))
    in_maps = []
    for k in range(N_CORES):
        m = dict(params)
        m.update({kk: vv for kk, vv in per_core[k].items() if not kk.startswith("_")})
        m["alpha1"] = alpha1[k]
        in_maps.append(m)
    return ncw, in_maps


def kernel(**inputs):
    _install_ntff_hook()
    from concourse.bass_utils import run_bass_kernel_spmd

    ncw, in_maps = make_in_maps(inputs)
    nc = build(ncw)
    res = run_bass_kernel_spmd(nc, in_maps, core_ids=list(range(N_CORES)), trace=False)
    out = np.concatenate([res.results[k]["out"] for k in range(N_CORES)], axis=0)
    return out.astype(np.float32)



# revision 30
# speedup vs baseline: 1.1376x; 1.0342x over previous
"""GATNet (4-layer GAT, 10000 nodes / 50000 edges + self-loops) on 8 Trainium2 NeuronCores.

Self-contained: builds per-core shards on the host (edge bucketing by destination,
one-hot scatter masks, gather index tables), compiles one SPMD Bass program, runs it
on cores 0-7 via run_bass_kernel_spmd, and reassembles the full [10000, 1000] output.

Structure per layer:
  dense h = y @ W^T (bf16, attention projections folded as extra rhs columns).
  The previous layer's graph-LN is folded in: diag(lnw) is baked into the rhs
  weights on the host; the runtime rinv scalar and the (lnb - mu*rinv*lnw)@W
  row const (broadcast via a K=1 matmul) are applied at the PSUM->SBUF
  eviction on the Vector engine, so the h table holds the exact LN'd h and
  the dense consumes raw (pre-LN) zT -- no LN apply pass, no stats on the
  dense critical path.
  -> small AllGather of the per-node attention scores (al); the softmax
     pre-phase (als gathers; ald via one-hot maskT matmuls from the local al
     shard; e-values, segment denominators, reciprocals, alphas) overlaps the
     dense; the two big half-table AllGathers are emitted after it (collective
     instructions block the gpsimd queue until completion, so the pre-phase
     gathers must be issued first)
  -> gather phase: dma_gather of source h rows; one-hot scatter matmuls with
     the per-edge alpha folded into the stationary operand; bias via K=1
     matmul; ReLU eviction (ACT, fused row sums) with all PSUM->SBUF copies on
     the Vector engine; PE-transpose into feature-major for the next lhsT
  -> graph-LN stats via tiny AllReduce -> (mu, rinv) broadcast + next biasT.
Layer 1 never materializes h: by linearity sum_e alpha_e * (x W)[src_e] =
(sum_e alpha_e x[src_e]) W, so it scatters raw x rows (256 wide) and applies W1
once per destination window; its attention scores are computed exactly on the host.
"""
import sys
import types

import numpy as np
import ml_dtypes

BF16 = ml_dtypes.bfloat16

N_NODES = 10000
N_CORES = 8
NPC = 1250
NPAD = 1280
NT = 10
NW = 10
HEADS_L = [8, 8, 8, 1]
C_L = [448, 384, 256, 1000]
FIN_L = [256, 3584, 3072, 2048]
FOUT_L = [3584, 3072, 2048, 1000]
TCOL_L = [256, 3072, 2048, 1024]    # bf16 columns of the gather table (L1: raw x)
EXP_CLAMP = 35.0
DEN_TINY = 1e-30


def _install_ntff_hook():
    if "antenv.axon_hooks" in sys.modules:
        return
    try:
        import antenv
        from trn_agent_boot.trn_boot import _ntff_profile_via_ctypes
    except ImportError:
        return
    mod = types.ModuleType("antenv.axon_hooks")
    state = {"hook": None}
    mod.set_axon_ntff_profile_hook = lambda h: state.__setitem__("hook", h)
    mod.get_axon_ntff_profile_hook = lambda: state["hook"]
    sys.modules["antenv.axon_hooks"] = mod
    antenv.axon_hooks = mod
    mod.set_axon_ntff_profile_hook(_ntff_profile_via_ctypes("/opt/axon/libaxon_pjrt.so"))


# ---------------------------------------------------------------- host prep
def _table_row(n):
    return NPAD * (n // NPC) + (n % NPC)


def _wrap16(idx_chunk):
    w = idx_chunk.reshape(8, 16).T
    return np.tile(w, (8, 1)).astype(np.int16)


def prep_edges(edge_index):
    src = np.asarray(edge_index[0], dtype=np.int64)
    dst = np.asarray(edge_index[1], dtype=np.int64)
    src = np.concatenate([src, np.arange(N_NODES, dtype=np.int64)])
    dst = np.concatenate([dst, np.arange(N_NODES, dtype=np.int64)])

    buckets = [[[] for _ in range(NW)] for _ in range(N_CORES)]
    core_of = dst // NPC
    win_of = (dst % NPC) // 128
    order = np.argsort(dst, kind="stable")
    for e in order:
        buckets[core_of[e]][win_of[e]].append(e)

    ncw = []
    for w in range(NW):
        mx = max(len(buckets[k][w]) for k in range(N_CORES))
        ncw.append(max(1, -(-mx // 128)))
    nch = sum(ncw)

    per_core = []
    for k in range(N_CORES):
        idx_s = np.zeros((128, nch * 8), np.int16)
        idx_d = np.zeros((128, nch * 8), np.int16)
        mask = np.zeros((128, nch, 128), np.float32)
        maskT = np.zeros((128, nch, 128), np.float32)
        esrc = np.zeros((nch, 128), np.int64)
        edst = np.zeros((nch, 128), np.int64)
        ereal = np.zeros((nch, 128), bool)
        c0 = 0
        for w in range(NW):
            edges = buckets[k][w]
            for c in range(ncw[w]):
                part = edges[c * 128:(c + 1) * 128]
                srows = np.zeros(128, np.int64)
                drows = np.zeros(128, np.int64)
                for i, e in enumerate(part):
                    srows[i] = _table_row(src[e])
                    drows[i] = _table_row(dst[e])
                    esrc[c0 + c, i] = src[e]
                    edst[c0 + c, i] = dst[e]
                    ereal[c0 + c, i] = True
                    d_local = (dst[e] % NPC) - 128 * w
                    mask[i, c0 + c, d_local] = 1.0
                    maskT[d_local, c0 + c, i] = 1.0
                idx_s[:, (c0 + c) * 8:(c0 + c + 1) * 8] = _wrap16(srows)
                idx_d[:, (c0 + c) * 8:(c0 + c + 1) * 8] = _wrap16(drows)
            c0 += ncw[w]
        per_core.append(dict(
            idxs=idx_s,
            mask=mask.reshape(128, nch * 128).astype(BF16),
            maskT=maskT.reshape(128, nch * 128).astype(BF16),
            _esrc=esrc, _edst=edst, _ereal=ereal,
        ))
    return tuple(ncw), per_core


def prep_alpha1(per_core, ncw, al1):
    """Exact layer-1 softmax on the host: alpha[e, h] per (chunk, slot)."""
    nch = sum(ncw)
    als = al1[:, :8].astype(np.float64)
    ald = al1[:, 8:].astype(np.float64)
    out = []
    for pc in per_core:
        esrc, edst, ereal = pc["_esrc"], pc["_edst"], pc["_ereal"]
        e = als[esrc] + ald[edst]                      # [nch, 128, 8]
        e = np.maximum(e, 0.2 * e)
        wv = np.exp(np.minimum(e, EXP_CLAMP)) * ereal[:, :, None]
        den = np.zeros((NPC, 8))
        np.add.at(den, (edst % NPC).reshape(-1), wv.reshape(-1, 8))
        alpha = wv / np.maximum(den[(edst % NPC)], 1e-300)
        # device layout: [part=slot, nch*8]
        return_arr = np.ascontiguousarray(
            alpha.transpose(1, 0, 2).reshape(128, nch * 8)).astype(BF16)
        out.append(return_arr)
    return out


def prep_params(inputs):
    p = {}
    x64 = np.asarray(inputs["x"], np.float64)
    al1 = None
    for li in range(4):
        H, C, fin, fout = HEADS_L[li], C_L[li], FIN_L[li], FOUT_L[li]
        W = np.asarray(inputs[f"W{li+1}"], np.float64)
        a_src = np.asarray(inputs[f"a_src{li+1}"], np.float64)
        a_dst = np.asarray(inputs[f"a_dst{li+1}"], np.float64)
        a_blk_s = np.zeros((fout, H), np.float64)
        a_blk_d = np.zeros((fout, H), np.float64)
        for h in range(H):
            a_blk_s[h * C:(h + 1) * C, h] = a_src[h]
            a_blk_d[h * C:(h + 1) * C, h] = a_dst[h]
        rhs = np.concatenate([W.T, W.T @ a_blk_s, W.T @ a_blk_d], axis=1)
        # graph-LN of the previous layer folded in: y = rinv*(z*lnw) + (lnb - mu*rinv*lnw)
        # so h = y@rhs = rinv*(z @ diag(lnw)@rhs) + r1 - (mu*rinv)*r2 with the
        # static diag(lnw) baked into the device rhs and r1/r2 host rows.
        if li > 0:
            lwp = np.asarray(inputs[f"ln{li}_w"], np.float64)
            lbp = np.asarray(inputs[f"ln{li}_b"], np.float64)
            p[f"rhs{li+1}"] = np.ascontiguousarray(lwp[:, None] * rhs).astype(BF16)
            p[f"r1_{li+1}"] = (lbp @ rhs).reshape(1, fout + 2 * H).astype(np.float32)
            p[f"r2_{li+1}"] = (lwp @ rhs).reshape(1, fout + 2 * H).astype(np.float32)
        else:
            p[f"rhs{li+1}"] = np.ascontiguousarray(rhs).astype(BF16)
        p[f"brow{li+1}"] = np.asarray(inputs[f"b{li+1}"], np.float32).reshape(1, fout).astype(BF16)
        if li == 0:
            al1 = np.concatenate(
                [x64 @ (W.T @ a_blk_s), x64 @ (W.T @ a_blk_d)], axis=1).astype(np.float32)
    # packed x table, replicated: [10240 rows, 768 bytes] = 512B x bf16 + 256B f32 al
    xtab = np.zeros((N_CORES * NPAD, 768), np.uint8)
    rows = _table_row(np.arange(N_NODES))
    xb = np.asarray(inputs["x"], np.float32).astype(BF16)
    xtab[rows, :512] = xb.view(np.uint8)
    alpad = np.zeros((N_NODES, 64), np.float32)
    alpad[:, :16] = al1
    xtab[rows, 512:768] = alpad.view(np.uint8)
    p["xtab"] = xtab.view(BF16)
    p["_al1"] = al1
    p["ident"] = np.eye(128, dtype=BF16)
    ones_b = np.zeros((1, NW * 128), np.float32)
    ones_b[0, :NPC] = 1.0
    p["ones_b"] = ones_b.astype(BF16)
    return p


# ---------------------------------------------------------------- device build
_CACHE = {}


def build(ncw, debug=False):
    key = (tuple(ncw), debug)
    if key in _CACHE:
        return _CACHE[key]

    import concourse.bacc as bacc
    import concourse.mybir as mybir
    import concourse.tile as tile
    from concourse.library_config import mlp

    f32 = mybir.dt.float32
    bf16 = mybir.dt.bfloat16
    i16 = mybir.dt.int16
    AX = mybir.AxisListType
    ALU = mybir.AluOpType
    ACTF = mybir.ActivationFunctionType

    nch = sum(ncw)
    ncmax = max(ncw)
    coff = [0]
    for w in range(NW):
        coff.append(coff[-1] + ncw[w])
    nc = bacc.Bacc("TRN2", num_swdge_queues=4)

    xtab_d = nc.declare_dram_parameter("xtab", [N_CORES * NPAD, 384], bf16, isOutput=False)
    rhs_d, brow_d, r1_d, r2_d = [], [], [None], [None]
    for li in range(4):
        H, fout, fin = HEADS_L[li], FOUT_L[li], FIN_L[li]
        rhs_d.append(nc.declare_dram_parameter(f"rhs{li+1}", [fin, fout + 2 * H], bf16, isOutput=False))
        brow_d.append(nc.declare_dram_parameter(f"brow{li+1}", [1, fout], bf16, isOutput=False))
        if li > 0:
            r1_d.append(nc.declare_dram_parameter(f"r1_{li+1}", [1, fout + 2 * H], f32, isOutput=False))
            r2_d.append(nc.declare_dram_parameter(f"r2_{li+1}", [1, fout + 2 * H], f32, isOutput=False))
    alpha1_d = nc.declare_dram_parameter("alpha1", [128, nch * 8], bf16, isOutput=False)
    idxs_d = nc.declare_dram_parameter("idxs", [128, nch * 8], i16, isOutput=False)
    mask_d = nc.declare_dram_parameter("mask", [128, nch * 128], bf16, isOutput=False)
    maskT_d = nc.declare_dram_parameter("maskT", [128, nch * 128], bf16, isOutput=False)
    ones_d = nc.declare_dram_parameter("ones_b", [1, NW * 128], bf16, isOutput=False)
    ident_d = nc.declare_dram_parameter("ident", [128, 128], bf16, isOutput=False)
    out_d = nc.declare_dram_parameter("out", [NPC, 1000], f32, isOutput=True)
    dbg_zt, dbg_st = [], []
    if debug:
        for li in range(3):
            dbg_zt.append(nc.declare_dram_parameter(
                f"dbg_zt{li}", [128, 28 * NPAD], bf16, isOutput=True))
            dbg_st.append(nc.declare_dram_parameter(
                f"dbg_st{li}", [1, 8], f32, isOutput=True))

    RG = [list(range(N_CORES))]

    with tile.TileContext(nc) as tc:
        with (
            tc.tile_pool(name="const", bufs=1) as constp,
            tc.tile_pool(name="yt", bufs=1) as ytp,
            tc.tile_pool(name="rhs", bufs=5) as rhsp,
            tc.tile_pool(name="stage", bufs=2) as stagep,
            tc.tile_pool(name="gath", bufs=2) as gathp,
            tc.tile_pool(name="mw", bufs=2) as mwp,
            tc.tile_pool(name="eph", bufs=2) as ephp,
            tc.tile_pool(name="z", bufs=2) as zp,
            tc.tile_pool(name="misc", bufs=2) as miscp,
            tc.tile_pool(name="dram", bufs=1, space="DRAM") as dram,
        ):
            nc.gpsimd.load_library(mlp)

            # warmup collective: the first CC op on the device pays a ~140us
            # one-time init; issue a tiny AllReduce up front so that cost
            # overlaps the L1 edge phase instead of the L1 LN AllReduce
            warm_sb = miscp.tile([1, 8], f32, tag="warm")
            nc.vector.memset(warm_sb[:], 1.0)
            warm_dram = dram.tile([1, 8], f32, tag="warmd")
            warm_glob = dram.tile([1, 8], f32, addr_space="Shared", tag="warmg")
            nc.sync.dma_start(warm_dram[:], warm_sb[:])
            nc.gpsimd.collective_compute(
                "AllReduce", ALU.add, ins=[warm_dram[:]], outs=[warm_glob[:]],
                replica_groups=RG)

            idxs_t = constp.tile([128, nch, 8], i16, tag="idxs")
            nc.sync.dma_start(idxs_t[:], idxs_d[:].rearrange("p (c d) -> p c d", c=nch))
            ones_t = constp.tile([1, NW, 128], bf16, tag="onesb")
            nc.sync.dma_start(ones_t[:], ones_d[:].rearrange("p (w d) -> p w d", w=NW))
            ident_t = constp.tile([128, 128], bf16, tag="ident")
            nc.sync.dma_start(ident_t[:], ident_d[:])
            ones128 = constp.tile([128, 1], f32, tag="ones128")
            nc.vector.memset(ones128[:], 1.0)
            onesT = constp.tile([1, 128], f32, tag="onesT")
            nc.vector.memset(onesT[:], 1.0)
            alpha1_t = constp.tile([128, nch, 8], bf16, tag="alpha1")
            nc.sync.dma_start(alpha1_t[:], alpha1_d[:].rearrange("p (c d) -> p c d", c=nch))
            # resident W1 rhs (small; needed per destination window in layer 1)
            rt1 = constp.tile([128, 2, 3584], bf16, tag="rt1")
            nc.sync.dma_start(
                rt1[:], rhs_d[0][:, 0:3584].rearrange("(k p) n -> p k n", p=128))

            yT = None  # produced by each layer's LN for the next layer
            pending_biasT = []  # deferred biasT chunk builders for the next dense

            biasT = None  # [128, fout+2H] f32: rinv-independent LN row consts for this dense
            br = None     # [128, 2] f32: (mu, rinv) of previous layer's graph-LN

            for li in range(4):
                H, C, fin, fout = HEADS_L[li], C_L[li], FIN_L[li], FOUT_L[li]
                tcol = TCOL_L[li]
                kch = fin // 128
                acols = 2 * H

                brow = constp.tile([1, 3584], bf16, tag="brow")
                nc.sync.dma_start(brow[:, :fout], brow_d[li][:])

                if li > 0:
                    # ===== dense: h = y @ W^T (+ al columns); al chunk FIRST so the
                    # small al AllGather + softmax pre-phase overlap the dense phase
                    half = [None, 2560, 1536, 512][li]
                    shard_a = dram.tile([NPAD, half], bf16, tag=f"sharda{li}")
                    shard_b = dram.tile([NPAD, tcol - half], bf16, tag=f"shardb{li}")
                    glob_a = dram.tile([N_CORES * NPAD, half], bf16, addr_space="Shared", tag=f"globa{li}")
                    glob_b = dram.tile([N_CORES * NPAD, tcol - half], bf16, addr_space="Shared", tag=f"globb{li}")
                    ashard = dram.tile([NPAD, 64], f32, tag=f"ashard{li}")
                    aglob = dram.tile([N_CORES * NPAD, 64], f32, addr_space="Shared", tag=f"aglob{li}")
                    fcs = [(fout, acols)]
                    o = 0
                    while o < fout:
                        w_ = min(512, fout - o)
                        fcs.append((o, w_))
                        o += w_
                    # fci index after which all shard_a columns are written
                    fci_a_done = 0
                    o = 0
                    for fci, (fo, fw) in enumerate(fcs):
                        if fci > 0 and fo + fw <= half:
                            fci_a_done = fci
                    with tc.tile_pool(name=f"psA{li}", bufs=4, space="PSUM") as mmp:
                        # ---- softmax pre-phase, emitted as pipelined stages
                        # interleaved between dense row-tiles so PE/vector/
                        # scalar work overlaps the dense instead of trailing it
                        al_f32 = aglob[:]
                        alpha_all = ephp.tile([128, nch, 8], bf16, tag="alpha", bufs=1)
                        wstate = {}

                        def s1a(w):
                            c0, ncwW = coff[w], ncw[w]
                            ne = ncwW * 128
                            As = ephp.tile([128, ncmax, 64], f32, tag="as", bufs=2)
                            nc.gpsimd.dma_gather(
                                As[:, :ncwW, :], al_f32,
                                idxs_t[:, c0:c0 + ncwW, :], ne, ne, 64, elem_step=64,
                                queue_num=(2 * w) % 4)
                            maskw = mwp.tile([128, ncmax, 128], bf16, tag="mw", bufs=2)
                            nc.scalar.dma_start(
                                maskw[:, :ncwW, :],
                                mask_d[:, c0 * 128:(c0 + ncwW) * 128].rearrange(
                                    "p (c d) -> p c d", c=ncwW))
                            maskTw = mwp.tile([128, ncmax, 128], bf16, tag="mwT", bufs=2)
                            nc.scalar.dma_start(
                                maskTw[:, :ncwW, :],
                                maskT_d[:, c0 * 128:(c0 + ncwW) * 128].rearrange(
                                    "p (c d) -> p c d", c=ncwW))
                            # ald of the window's own dst nodes, scattered to edge
                            # slots via the one-hot maskT (replaces a dma_gather)
                            ald_f = ephp.tile([128, 8], f32, tag="aldf", bufs=3)
                            nc.sync.dma_start(
                                ald_f[:, :H], ashard[w * 128:(w + 1) * 128, H:2 * H])
                            ald_b = ephp.tile([128, 8], bf16, tag="aldb", bufs=3)
                            nc.vector.tensor_copy(ald_b[:, :H], ald_f[:, :H])
                            wstate[w] = (As, maskw, maskTw, ald_b)

                        def s1b(w):
                            c0, ncwW = coff[w], ncw[w]
                            As, maskw, maskTw, ald_b = wstate[w]
                            ps_ad = mmp.tile([128, ncmax, 8], f32, tag="smAd", bufs=2)
                            for c in range(ncwW):
                                nc.tensor.matmul(
                                    ps_ad[:, c, :H], maskTw[:, c, :], ald_b[:, :H],
                                    start=True, stop=True)
                            ev = ephp.tile([128, ncmax, 8], f32, tag="ev", bufs=2)
                            nc.vector.tensor_tensor(
                                ev[:, :ncwW, :H], As[:, :ncwW, 0:H], ps_ad[:, :ncwW, :H], ALU.add)
                            nc.vector.scalar_tensor_tensor(
                                ev[:, :ncwW, :H], ev[:, :ncwW, :H], 0.2, ev[:, :ncwW, :H],
                                ALU.mult, ALU.max)
                            nc.vector.tensor_scalar_min(ev[:, :ncwW, :H], ev[:, :ncwW, :H], EXP_CLAMP)
                            wv = ephp.tile([128, ncmax, 8], bf16, tag="wv", bufs=2)
                            nc.scalar.activation(wv[:, :ncwW, :H], ev[:, :ncwW, :H], ACTF.Exp)
                            wstate[w] = (wv, maskw, maskTw)

                        def s2(w):
                            c0, ncwW = coff[w], ncw[w]
                            wv, maskw, maskTw = wstate[w]
                            ps_den = mmp.tile([128, 8], f32, tag="smA", bufs=2)
                            for c in range(ncwW):
                                nc.tensor.matmul(
                                    ps_den[:, :H], maskw[:, c, :], wv[:, c, :H],
                                    start=(c == 0), stop=(c == ncwW - 1))
                            rden_f = ephp.tile([128, 8], f32, tag="rdenf", bufs=2)
                            nc.vector.tensor_scalar_max(rden_f[:, :H], ps_den[:, :H], DEN_TINY)
                            rden2 = ephp.tile([128, 8], f32, tag="rden2", bufs=2)
                            nc.vector.reciprocal(rden2[:, :H], rden_f[:, :H])
                            rden = ephp.tile([128, 8], bf16, tag="rden", bufs=2)
                            nc.vector.tensor_copy(rden[:, :H], rden2[:, :H])
                            wstate[w] = (wv, maskw, maskTw, rden)

                        def s3(w):
                            c0, ncwW = coff[w], ncw[w]
                            wv, maskw, maskTw, rden = wstate.pop(w)
                            # batched: all chunk matmuls into one PSUM tile, then a
                            # single vector mult -- no PE<->vector ping-pong stalls
                            ps_exp = mmp.tile([128, ncmax, 8], f32, tag="smAd", bufs=2)
                            for c in range(ncwW):
                                nc.tensor.matmul(
                                    ps_exp[:, c, :H], maskTw[:, c, :], rden[:, :H],
                                    start=True, stop=True)
                            nc.vector.tensor_tensor(
                                alpha_all[:, c0:c0 + ncwW, :H], wv[:, :ncwW, :H],
                                ps_exp[:, :ncwW, :H], ALU.mult)

                        qa = [(s1a, w) for w in range(NW)]
                        qb = [(f, w) for w in range(NW) for f in (s1b, s2, s3)]
                        na = nb = 0

                        def pop_stage():
                            nonlocal na, nb
                            if qa and na < nb // 3 + 3:
                                f, w = qa.pop(0); na += 1
                            elif qb and (nb // 3 + 2 <= na or not qa):
                                f, w = qb.pop(0); nb += 1
                            elif qa:
                                f, w = qa.pop(0); na += 1
                            else:
                                return False
                            f(w)
                            return True

                        per_slot = 2 if li == 3 else 1
                        slot = 0
                        for fci, (fo, fw) in enumerate(fcs):
                            kgrps = [(k0, min(7, kch - k0)) for k0 in range(0, kch, 7)]
                            rts = []
                            for (k0, kn) in kgrps:
                                rt = rhsp.tile([128, 7, 512], bf16, tag="rhs")
                                nc.sync.dma_start(
                                    rt[:, :kn, :fw],
                                    rhs_d[li][k0 * 128:(k0 + kn) * 128, fo:fo + fw]
                                    .rearrange("(k p) n -> p k n", p=128))
                                rts.append(rt)
                            for t in range(NT):
                                # deferred biasT chunks first: the eviction below
                                # reads biasT, so its writers must precede it
                                if fci >= 1:
                                    for _ in range(2):
                                        if pending_biasT:
                                            pending_biasT.pop(0)(mmp)
                                ps = mmp.tile([128, 512], f32, tag="mm")
                                for kc in range(kch):
                                    nc.tensor.matmul(
                                        ps[:, :fw],
                                        yT[:, kc, t * 128:(t + 1) * 128],
                                        rts[kc // 7][:, kc % 7, :fw],
                                        start=(kc == 0),
                                        stop=(kc == kch - 1))
                                hw = max(0, min(fw, fout - fo))
                                if hw > 0:
                                    # h' = rinv*(z.lnw @ W) + (lnb - mu*rinv*lnw)@W : the
                                    # table rows carry the exact LN'd h (see prep_params)
                                    st = stagep.tile([128, 512], bf16, tag="stg")
                                    nc.vector.scalar_tensor_tensor(
                                        st[:, :hw], ps[:, :hw], br[:, 1:2],
                                        biasT[:, fo:fo + hw], ALU.mult, ALU.subtract)
                                    if fo < half:
                                        nc.sync.dma_start(
                                            shard_a[t * 128:(t + 1) * 128, fo:fo + hw], st[:, :hw])
                                    else:
                                        nc.sync.dma_start(
                                            shard_b[t * 128:(t + 1) * 128, fo - half:fo - half + hw],
                                            st[:, :hw])
                                if hw < fw:
                                    a0 = fo + hw - fout
                                    sa = stagep.tile([128, 16], f32, tag="stga")
                                    nc.vector.scalar_tensor_tensor(
                                        sa[:, :fw - hw], ps[:, hw:fw], br[:, 1:2],
                                        biasT[:, fout + a0:fout + a0 + fw - hw],
                                        ALU.mult, ALU.subtract)
                                    nc.sync.dma_start(
                                        ashard[t * 128:(t + 1) * 128, a0:a0 + fw - hw],
                                        sa[:, :fw - hw])
                                if fci >= 1:
                                    if slot >= 2:
                                        for _ in range(per_slot):
                                            pop_stage()
                                    slot += 1
                            if fci == 0:
                                nc.gpsimd.collective_compute(
                                    "AllGather", ALU.bypass, ins=[ashard[:]], outs=[aglob[:]],
                                    replica_groups=RG)
                            if fci == fci_a_done:
                                # drain s1a gathers first: they must hit the gpsimd
                                # queue before the collective blocks it
                                while qa:
                                    pop_stage()
                                nc.gpsimd.collective_compute(
                                    "AllGather", ALU.bypass, ins=[shard_a[:]], outs=[glob_a[:]],
                                    replica_groups=RG)
                        while qa or qb:
                            pop_stage()

                    nc.gpsimd.collective_compute(
                        "AllGather", ALU.bypass, ins=[shard_b[:]], outs=[glob_b[:]],
                        replica_groups=RG)
                else:
                    alpha_all = alpha1_t

                # ===== edge phase: for li>0 two passes over column halves so
                # the glob_b AllGather hides under pass A's gather+scatter work
                edgeps = tc.tile_pool(name=f"psB{li}", bufs=1, space="PSUM")
                edgep = edgeps.__enter__()
                if li < 3:
                    zT = ytp.tile([128, 28, NPAD], bf16, tag="yt")
                    stats = miscp.tile([128, 2 * NW], f32, tag="stats")

                # ---- gather + scatter phase (single pass; glob_a covers cols
                # [0:half] with half > tcol/2 so the trailing AG_b is small)
                for w in range(NW):
                    c0, ncwW = coff[w], ncw[w]
                    maskw = mwp.tile([128, ncmax, 128], bf16, tag="mw", bufs=2)
                    nc.scalar.dma_start(
                        maskw[:, :ncwW, :],
                        mask_d[:, c0 * 128:(c0 + ncwW) * 128].rearrange(
                            "p (c d) -> p c d", c=ncwW))

                    if li == 0:
                        ps_agg = edgep.tile([128, 2, 8, 128], f32, tag="out")
                    else:
                        ps_out = edgep.tile([128, fout], f32, tag="out")

                    for cp in range(0, ncwW, 2):
                        cw = min(2, ncwW - cp)
                        # flat gather tiles: one shared tag for all layers/widths
                        if li == 0:
                            G = gathp.tile([128, 5120], bf16, tag="G")
                            nc.gpsimd.dma_gather(
                                G[:, :cw * 256].rearrange("p (c g) -> p c g", g=256),
                                xtab_d[:][:, 0:256],
                                idxs_t[:, c0 + cp:c0 + cp + cw, :], cw * 128, cw * 128,
                                256, elem_step=384, queue_num=(cp // 2) % 4)
                        else:
                            G = gathp.tile([128, 5120], bf16, tag="G")
                            nc.gpsimd.dma_gather(
                                G[:, :cw * half].rearrange("p (c g) -> p c g", g=half),
                                glob_a[:],
                                idxs_t[:, c0 + cp:c0 + cp + cw, :], cw * 128, cw * 128,
                                half, elem_step=half, queue_num=(cp // 2) % 4)
                            Gb = gathp.tile([128, 1024], bf16, tag="Gb")
                            nc.gpsimd.dma_gather(
                                Gb[:, :cw * (tcol - half)].rearrange("p (c g) -> p c g", g=tcol - half),
                                glob_b[:],
                                idxs_t[:, c0 + cp:c0 + cp + cw, :], cw * 128, cw * 128,
                                tcol - half, elem_step=tcol - half,
                                queue_num=(cp // 2 + 2) % 4)
                        for c in range(cp, cp + cw):
                            lhs = ephp.tile([128, 8, 128], bf16, tag="lhs")
                            nc.vector.tensor_tensor(
                                lhs[:, :H, :],
                                maskw[:, c, :].unsqueeze(1).broadcast_to([128, H, 128]),
                                alpha_all[:, c0 + c, :H].unsqueeze(2).broadcast_to([128, H, 128]),
                                ALU.mult)
                            if li == 0:
                                gof = (c - cp) * 256
                                for kc in range(2):
                                    for hg in range(2):
                                        nc.tensor.matmul(
                                            ps_agg[:, kc, hg * 4:(hg + 1) * 4, :],
                                            G[:, gof + kc * 128:gof + (kc + 1) * 128],
                                            lhs[:, hg * 4:(hg + 1) * 4, :],
                                            start=(c == 0), stop=False)
                            else:
                                o = 0
                                while o < fout:
                                    h = o // C
                                    e = min((h + 1) * C, (o // 512 + 1) * 512, fout)
                                    if o < half:
                                        e = min(e, half)
                                        gof = (c - cp) * half
                                        rhs_g = G[:, gof + o:gof + e]
                                    else:
                                        bof = (c - cp) * (tcol - half)
                                        rhs_g = Gb[:, bof + o - half:bof + e - half]
                                    nc.tensor.matmul(
                                        ps_out[:, o:e], lhs[:, h, :], rhs_g,
                                        start=(c == 0 and o % 512 == 0), stop=False)
                                    o = e

                    if li == 0:
                        # xaggT came out of the scatter matmuls already transposed
                        xs = zp.tile([128, 2, 8, 128], bf16, tag="z")
                        nc.vector.tensor_copy(xs[:], ps_agg[:])
                        ps_out = edgep.tile([128, fout], f32, tag="out")
                        for h in range(H):
                            o = h * C
                            while o < (h + 1) * C:
                                e = min((o // 512 + 1) * 512, (h + 1) * C)
                                for kc in range(2):
                                    nc.tensor.matmul(
                                        ps_out[:, o:e], xs[:, kc, h, :],
                                        rt1[:, kc, o:e],
                                        start=(kc == 0 and o % 512 == 0), stop=False)
                                o = e
                    # bias add
                    o = 0
                    while o < fout:
                        e = min(o + 512, fout)
                        nc.tensor.matmul(
                            ps_out[:, o:e], ones_t[:, w, :], brow[:, o:e],
                            start=False, stop=(e == fout))
                        o = e

                    if li < 3:
                        z = zp.tile([128, 3584], bf16, tag="z")
                        nc.scalar.activation(
                            z[:, :fout], ps_out[:, :fout], ACTF.Relu,
                            accum_out=stats[:, w:w + 1])
                        sq = zp.tile([128, 3584], bf16, tag="z")
                        nc.vector.scalar_tensor_tensor(
                            sq[:, :fout], z[:, :fout], 1.0, z[:, :fout],
                            ALU.mult, ALU.mult,
                            accum_out=stats[:, NW + w:NW + w + 1])
                        for q in range(0, fout // 128, 4):
                            qn = min(4, fout // 128 - q)
                            ps_t = edgep.tile([128, 4, 128], bf16, tag="sm")
                            for j in range(qn):
                                nc.tensor.matmul(
                                    ps_t[:, j, :], z[:, (q + j) * 128:(q + j + 1) * 128],
                                    ident_t[:], is_transpose=True,
                                    start=(j == 0), stop=(j == qn - 1))
                            nc.vector.tensor_copy(
                                zT[:, q:q + qn, w * 128:(w + 1) * 128],
                                ps_t[:, :qn, :])
                    else:
                        zf = zp.tile([128, 1024], f32, tag="z")
                        nc.vector.tensor_copy(zf[:, :fout], ps_out[:, :fout])
                        rows = min(128, NPC - w * 128)
                        nc.sync.dma_start(out_d[w * 128:w * 128 + rows, :], zf[:rows, :fout])

                # ===== graph LayerNorm + next yT
                if li < 3:
                    sdram = dram.tile([1, 64], f32, tag=f"sd{li}")
                    sglob = dram.tile([1, 64], f32, addr_space="Shared", tag=f"sg{li}")
                    ps_s = edgep.tile([1, 2 * NW], f32, tag="sm")
                    nc.tensor.matmul(ps_s[:], ones128[:], stats[:], start=True, stop=True)
                    ssum = miscp.tile([1, 4], f32, tag="ssum")
                    nc.vector.tensor_reduce(ssum[:, 0:1], ps_s[:, 0:NW], AX.X, ALU.add)
                    nc.vector.tensor_reduce(ssum[:, 1:2], ps_s[:, NW:2 * NW], AX.X, ALU.add)
                    nc.sync.dma_start(sdram[:, 0:2], ssum[:, 0:2])
                    nc.gpsimd.collective_compute(
                        "AllReduce", ALU.add, ins=[sdram[:]], outs=[sglob[:]],
                        replica_groups=RG)
                    gsum = miscp.tile([1, 8], f32, tag="gsum")
                    nc.sync.dma_start(gsum[:, 0:2], sglob[:, 0:2])
                    sc = miscp.tile([1, 8], f32, tag="sc")
                    inv_cnt = 1.0 / (float(N_NODES) * fout)
                    nc.vector.tensor_scalar_mul(sc[:, 0:2], gsum[:, 0:2], inv_cnt)
                    nc.vector.tensor_tensor(sc[:, 2:3], sc[:, 0:1], sc[:, 0:1], ALU.mult)
                    nc.vector.tensor_sub(sc[:, 3:4], sc[:, 1:2], sc[:, 2:3])
                    nc.vector.tensor_scalar_add(sc[:, 3:4], sc[:, 3:4], 1e-5)
                    nc.scalar.sqrt(sc[:, 4:5], sc[:, 3:4])
                    nc.vector.reciprocal(sc[:, 5:6], sc[:, 4:5])
                    mr = miscp.tile([1, 2], f32, tag="mr")
                    nc.vector.tensor_copy(mr[:, 0:1], sc[:, 0:1])
                    nc.vector.tensor_copy(mr[:, 1:2], sc[:, 5:6])
                    ps_b = edgep.tile([128, 2], f32, tag="sm")
                    nc.tensor.matmul(ps_b[:], onesT[:], mr[:], start=True, stop=True)
                    br = miscp.tile([128, 2], f32, tag=f"br{li}")
                    nc.vector.tensor_copy(br[:], ps_b[:])
                    c128 = miscp.tile([128, 1], f32, tag=f"c{li}")
                    nc.vector.tensor_tensor(c128[:], br[:, 0:1], br[:, 1:2], ALU.mult)
                    # next dense's rinv-independent row consts, negated:
                    # biasT = (mu*rinv)*r2 - r1  (evictions subtract it).
                    # Only the al-columns chunk is built here (the next dense's
                    # fci=0 evictions read it immediately); the rest is deferred
                    # into the next dense loop, hidden under its matmuls.
                    wnx = FOUT_L[li + 1] + 2 * HEADS_L[li + 1]
                    wnx0 = (FOUT_L[li + 1] // 256) * 256
                    biasT = miscp.tile([128, 3088], f32, tag="biasT", bufs=1)

                    def bt_chunk(o, e, pool, tag, pbufs=1, c128=c128, li=li, biasT=biasT):
                        r2row = miscp.tile([1, 256], f32, tag="r2row", bufs=2)
                        nc.sync.dma_start(r2row[:, :e - o], r2_d[li + 1][:, o:e])
                        r1row = miscp.tile([1, 256], f32, tag="r1row", bufs=2)
                        nc.sync.dma_start(r1row[:, :e - o], r1_d[li + 1][:, o:e])
                        rrow = miscp.tile([1, 256], f32, tag="rrow", bufs=2)
                        nc.vector.scalar_tensor_tensor(
                            rrow[:, :e - o], r2row[:, :e - o], c128[0:1, :],
                            r1row[:, :e - o], ALU.mult, ALU.subtract)
                        ps_bt = pool.tile([128, 256], f32, tag=tag, bufs=pbufs)
                        nc.tensor.matmul(
                            ps_bt[:, :e - o], onesT[:], rrow[:, :e - o],
                            start=True, stop=True)
                        nc.vector.tensor_copy(biasT[:, o:e], ps_bt[:, :e - o])

                    bt_chunk(wnx0, wnx, edgep, "sm")
                    o = 0
                    while o < wnx0:
                        e = min(o + 256, wnx0)
                        pending_biasT.append(
                            lambda pool, o=o, e=e: bt_chunk(o, e, pool, "smA", pbufs=2))
                        o = e
                    yT = zT
                    if debug:
                        nc.sync.dma_start(
                            dbg_zt[li][:], zT[:].rearrange("p q n -> p (q n)"))
                        nc.sync.dma_start(dbg_st[li][:], sc[:])
                edgeps.__exit__(None, None, None)

    nc.compile()
    _CACHE[key] = nc
    return nc


# ---------------------------------------------------------------- entry point
def make_in_maps(inputs):
    ncw, per_core = prep_edges(inputs["edge_index"])
    params = prep_params(inputs)
    alpha1 = prep_alpha1(per_core, ncw, params.pop("_al1"))
    in_maps = []
    for k in range(N_CORES):
        m = dict(params)
        m.update({kk: vv for kk, vv in per_core[k].items() if not kk.startswith("_")})
        m["alpha1"] = alpha1[k]
        in_maps.append(m)
    return ncw, in_maps


def kernel(**inputs):
    _install_ntff_hook()
    from concourse.bass_utils import run_bass_kernel_spmd

    ncw, in_maps = make_in_maps(inputs)
    nc = build(ncw)
    res = run_bass_kernel_spmd(nc, in_maps, core_ids=list(range(N_CORES)), trace=False)
    out = np.concatenate([res.results[k]["out"] for k in range(N_CORES)], axis=0)
    return out.astype(np.float32)



# revision 32
# speedup vs baseline: 1.1577x; 1.0177x over previous
"""GATNet (4-layer GAT, 10000 nodes / 50000 edges + self-loops) on 8 Trainium2 NeuronCores.

Self-contained: builds per-core shards on the host (edge bucketing by destination,
one-hot scatter masks, gather index tables), compiles one SPMD Bass program, runs it
on cores 0-7 via run_bass_kernel_spmd, and reassembles the full [10000, 1000] output.

Structure per layer:
  dense h = y @ W^T (bf16, attention projections folded as extra rhs columns).
  The previous layer's graph-LN is folded in: diag(lnw) is baked into the rhs
  weights on the host; the runtime rinv scalar and the (lnb - mu*rinv*lnw)@W
  row const (broadcast via a K=1 matmul) are applied at the PSUM->SBUF
  eviction on the Vector engine, so the h table holds the exact LN'd h and
  the dense consumes raw (pre-LN) zT -- no LN apply pass, no stats on the
  dense critical path.
  -> small AllGather of the per-node attention scores (al); the softmax
     pre-phase (als gathers; ald via one-hot maskT matmuls from the local al
     shard; e-values, segment denominators, reciprocals, alphas) overlaps the
     dense; the two big half-table AllGathers are emitted after it (collective
     instructions block the gpsimd queue until completion, so the pre-phase
     gathers must be issued first)
  -> gather phase: dma_gather of source h rows; one-hot scatter matmuls with
     the per-edge alpha folded into the stationary operand; bias via K=1
     matmul; ReLU eviction (ACT, fused row sums) with all PSUM->SBUF copies on
     the Vector engine; PE-transpose into feature-major for the next lhsT
  -> graph-LN stats via tiny AllReduce -> (mu, rinv) broadcast + next biasT.
Layer 1 never materializes h: by linearity sum_e alpha_e * (x W)[src_e] =
(sum_e alpha_e x[src_e]) W, so it scatters raw x rows (256 wide) and applies W1
once per destination window; its attention scores are computed exactly on the host.
"""
import sys
import types

import numpy as np
import ml_dtypes

BF16 = ml_dtypes.bfloat16

N_NODES = 10000
N_CORES = 8
NPC = 1250
NPAD = 1280
NT = 10
NW = 10
HEADS_L = [8, 8, 8, 1]
C_L = [448, 384, 256, 1000]
FIN_L = [256, 3584, 3072, 2048]
FOUT_L = [3584, 3072, 2048, 1000]
TCOL_L = [256, 3072, 2048, 1024]    # bf16 columns of the gather table (L1: raw x)
EXP_CLAMP = 35.0
DEN_TINY = 1e-30


def _install_ntff_hook():
    if "antenv.axon_hooks" in sys.modules:
        return
    try:
        import antenv
        from trn_agent_boot.trn_boot import _ntff_profile_via_ctypes
    except ImportError:
        return
    mod = types.ModuleType("antenv.axon_hooks")
    state = {"hook": None}
    mod.set_axon_ntff_profile_hook = lambda h: state.__setitem__("hook", h)
    mod.get_axon_ntff_profile_hook = lambda: state["hook"]
    sys.modules["antenv.axon_hooks"] = mod
    antenv.axon_hooks = mod
    mod.set_axon_ntff_profile_hook(_ntff_profile_via_ctypes("/opt/axon/libaxon_pjrt.so"))


# ---------------------------------------------------------------- host prep
def _table_row(n):
    return NPAD * (n // NPC) + (n % NPC)


def _wrap16(idx_chunk):
    w = idx_chunk.reshape(8, 16).T
    return np.tile(w, (8, 1)).astype(np.int16)


def prep_edges(edge_index):
    src = np.asarray(edge_index[0], dtype=np.int64)
    dst = np.asarray(edge_index[1], dtype=np.int64)
    src = np.concatenate([src, np.arange(N_NODES, dtype=np.int64)])
    dst = np.concatenate([dst, np.arange(N_NODES, dtype=np.int64)])

    buckets = [[[] for _ in range(NW)] for _ in range(N_CORES)]
    core_of = dst // NPC
    win_of = (dst % NPC) // 128
    order = np.argsort(dst, kind="stable")
    for e in order:
        buckets[core_of[e]][win_of[e]].append(e)

    ncw = []
    for w in range(NW):
        mx = max(len(buckets[k][w]) for k in range(N_CORES))
        ncw.append(max(1, -(-mx // 128)))
    nch = sum(ncw)

    per_core = []
    for k in range(N_CORES):
        idx_s = np.zeros((128, nch * 8), np.int16)
        idx_d = np.zeros((128, nch * 8), np.int16)
        mask = np.zeros((128, nch, 128), np.float32)
        maskT = np.zeros((128, nch, 128), np.float32)
        esrc = np.zeros((nch, 128), np.int64)
        edst = np.zeros((nch, 128), np.int64)
        ereal = np.zeros((nch, 128), bool)
        c0 = 0
        for w in range(NW):
            edges = buckets[k][w]
            for c in range(ncw[w]):
                part = edges[c * 128:(c + 1) * 128]
                srows = np.zeros(128, np.int64)
                drows = np.zeros(128, np.int64)
                for i, e in enumerate(part):
                    srows[i] = _table_row(src[e])
                    drows[i] = _table_row(dst[e])
                    esrc[c0 + c, i] = src[e]
                    edst[c0 + c, i] = dst[e]
                    ereal[c0 + c, i] = True
                    d_local = (dst[e] % NPC) - 128 * w
                    mask[i, c0 + c, d_local] = 1.0
                    maskT[d_local, c0 + c, i] = 1.0
                idx_s[:, (c0 + c) * 8:(c0 + c + 1) * 8] = _wrap16(srows)
                idx_d[:, (c0 + c) * 8:(c0 + c + 1) * 8] = _wrap16(drows)
            c0 += ncw[w]
        per_core.append(dict(
            idxs=idx_s,
            mask=mask.reshape(128, nch * 128).astype(BF16),
            maskT=maskT.reshape(128, nch * 128).astype(BF16),
            _esrc=esrc, _edst=edst, _ereal=ereal,
        ))
    return tuple(ncw), per_core


def prep_alpha1(per_core, ncw, al1):
    """Exact layer-1 softmax on the host: alpha[e, h] per (chunk, slot)."""
    nch = sum(ncw)
    als = al1[:, :8].astype(np.float64)
    ald = al1[:, 8:].astype(np.float64)
    out = []
    for pc in per_core:
        esrc, edst, ereal = pc["_esrc"], pc["_edst"], pc["_ereal"]
        e = als[esrc] + ald[edst]                      # [nch, 128, 8]
        e = np.maximum(e, 0.2 * e)
        wv = np.exp(np.minimum(e, EXP_CLAMP)) * ereal[:, :, None]
        den = np.zeros((NPC, 8))
        np.add.at(den, (edst % NPC).reshape(-1), wv.reshape(-1, 8))
        alpha = wv / np.maximum(den[(edst % NPC)], 1e-300)
        # device layout: [part=slot, nch*8]
        return_arr = np.ascontiguousarray(
            alpha.transpose(1, 0, 2).reshape(128, nch * 8)).astype(BF16)
        out.append(return_arr)
    return out


def prep_params(inputs):
    p = {}
    x64 = np.asarray(inputs["x"], np.float64)
    al1 = None
    for li in range(4):
        H, C, fin, fout = HEADS_L[li], C_L[li], FIN_L[li], FOUT_L[li]
        W = np.asarray(inputs[f"W{li+1}"], np.float64)
        a_src = np.asarray(inputs[f"a_src{li+1}"], np.float64)
        a_dst = np.asarray(inputs[f"a_dst{li+1}"], np.float64)
        a_blk_s = np.zeros((fout, H), np.float64)
        a_blk_d = np.zeros((fout, H), np.float64)
        for h in range(H):
            a_blk_s[h * C:(h + 1) * C, h] = a_src[h]
            a_blk_d[h * C:(h + 1) * C, h] = a_dst[h]
        rhs = np.concatenate([W.T, W.T @ a_blk_s, W.T @ a_blk_d], axis=1)
        # graph-LN of the previous layer folded in: y = rinv*(z*lnw) + (lnb - mu*rinv*lnw)
        # so h = y@rhs = rinv*(z @ diag(lnw)@rhs) + r1 - (mu*rinv)*r2 with the
        # static diag(lnw) baked into the device rhs and r1/r2 host rows.
        if li > 0:
            lwp = np.asarray(inputs[f"ln{li}_w"], np.float64)
            lbp = np.asarray(inputs[f"ln{li}_b"], np.float64)
            p[f"rhs{li+1}"] = np.ascontiguousarray(lwp[:, None] * rhs).astype(BF16)
            p[f"r1_{li+1}"] = (lbp @ rhs).reshape(1, fout + 2 * H).astype(np.float32)
            p[f"r2_{li+1}"] = (lwp @ rhs).reshape(1, fout + 2 * H).astype(np.float32)
        else:
            p[f"rhs{li+1}"] = np.ascontiguousarray(rhs).astype(BF16)
        p[f"brow{li+1}"] = np.asarray(inputs[f"b{li+1}"], np.float32).reshape(1, fout).astype(BF16)
        if li == 0:
            al1 = np.concatenate(
                [x64 @ (W.T @ a_blk_s), x64 @ (W.T @ a_blk_d)], axis=1).astype(np.float32)
    # packed x table, replicated: [10240 rows, 768 bytes] = 512B x bf16 + 256B f32 al
    xtab = np.zeros((N_CORES * NPAD, 768), np.uint8)
    rows = _table_row(np.arange(N_NODES))
    xb = np.asarray(inputs["x"], np.float32).astype(BF16)
    xtab[rows, :512] = xb.view(np.uint8)
    alpad = np.zeros((N_NODES, 64), np.float32)
    alpad[:, :16] = al1
    xtab[rows, 512:768] = alpad.view(np.uint8)
    p["xtab"] = xtab.view(BF16)
    p["_al1"] = al1
    p["ident"] = np.eye(128, dtype=BF16)
    ones_b = np.zeros((1, NW * 128), np.float32)
    ones_b[0, :NPC] = 1.0
    p["ones_b"] = ones_b.astype(BF16)
    return p


# ---------------------------------------------------------------- device build
_CACHE = {}


def build(ncw, debug=False):
    key = (tuple(ncw), debug)
    if key in _CACHE:
        return _CACHE[key]

    import concourse.bacc as bacc
    import concourse.mybir as mybir
    import concourse.tile as tile
    from concourse.library_config import mlp

    f32 = mybir.dt.float32
    bf16 = mybir.dt.bfloat16
    i16 = mybir.dt.int16
    AX = mybir.AxisListType
    ALU = mybir.AluOpType
    ACTF = mybir.ActivationFunctionType

    nch = sum(ncw)
    ncmax = max(ncw)
    coff = [0]
    for w in range(NW):
        coff.append(coff[-1] + ncw[w])
    nc = bacc.Bacc("TRN2", num_swdge_queues=4)

    xtab_d = nc.declare_dram_parameter("xtab", [N_CORES * NPAD, 384], bf16, isOutput=False)
    rhs_d, brow_d, r1_d, r2_d = [], [], [None], [None]
    for li in range(4):
        H, fout, fin = HEADS_L[li], FOUT_L[li], FIN_L[li]
        rhs_d.append(nc.declare_dram_parameter(f"rhs{li+1}", [fin, fout + 2 * H], bf16, isOutput=False))
        brow_d.append(nc.declare_dram_parameter(f"brow{li+1}", [1, fout], bf16, isOutput=False))
        if li > 0:
            r1_d.append(nc.declare_dram_parameter(f"r1_{li+1}", [1, fout + 2 * H], f32, isOutput=False))
            r2_d.append(nc.declare_dram_parameter(f"r2_{li+1}", [1, fout + 2 * H], f32, isOutput=False))
    alpha1_d = nc.declare_dram_parameter("alpha1", [128, nch * 8], bf16, isOutput=False)
    idxs_d = nc.declare_dram_parameter("idxs", [128, nch * 8], i16, isOutput=False)
    mask_d = nc.declare_dram_parameter("mask", [128, nch * 128], bf16, isOutput=False)
    maskT_d = nc.declare_dram_parameter("maskT", [128, nch * 128], bf16, isOutput=False)
    ones_d = nc.declare_dram_parameter("ones_b", [1, NW * 128], bf16, isOutput=False)
    ident_d = nc.declare_dram_parameter("ident", [128, 128], bf16, isOutput=False)
    out_d = nc.declare_dram_parameter("out", [NPC, 1000], f32, isOutput=True)
    dbg_zt, dbg_st = [], []
    if debug:
        for li in range(3):
            dbg_zt.append(nc.declare_dram_parameter(
                f"dbg_zt{li}", [128, 28 * NPAD], bf16, isOutput=True))
            dbg_st.append(nc.declare_dram_parameter(
                f"dbg_st{li}", [1, 8], f32, isOutput=True))

    RG = [list(range(N_CORES))]

    with tile.TileContext(nc) as tc:
        with (
            tc.tile_pool(name="const", bufs=1) as constp,
            tc.tile_pool(name="yt", bufs=1) as ytp,
            tc.tile_pool(name="rhs", bufs=5) as rhsp,
            tc.tile_pool(name="stage", bufs=2) as stagep,
            tc.tile_pool(name="gath", bufs=2) as gathp,
            tc.tile_pool(name="mw", bufs=2) as mwp,
            tc.tile_pool(name="eph", bufs=2) as ephp,
            tc.tile_pool(name="z", bufs=2) as zp,
            tc.tile_pool(name="misc", bufs=2) as miscp,
            tc.tile_pool(name="dram", bufs=1, space="DRAM") as dram,
        ):
            nc.gpsimd.load_library(mlp)

            # warmup collective: the first CC op on the device pays a ~140us
            # one-time init; issue a tiny AllReduce up front so that cost
            # overlaps the L1 edge phase instead of the L1 LN AllReduce
            warm_sb = miscp.tile([1, 8], f32, tag="warm")
            nc.vector.memset(warm_sb[:], 1.0)
            warm_dram = dram.tile([1, 8], f32, tag="warmd")
            warm_glob = dram.tile([1, 8], f32, addr_space="Shared", tag="warmg")
            nc.sync.dma_start(warm_dram[:], warm_sb[:])
            nc.gpsimd.collective_compute(
                "AllReduce", ALU.add, ins=[warm_dram[:]], outs=[warm_glob[:]],
                replica_groups=RG)

            idxs_t = constp.tile([128, nch, 8], i16, tag="idxs")
            nc.sync.dma_start(idxs_t[:], idxs_d[:].rearrange("p (c d) -> p c d", c=nch))
            ones_t = constp.tile([1, NW, 128], bf16, tag="onesb")
            nc.sync.dma_start(ones_t[:], ones_d[:].rearrange("p (w d) -> p w d", w=NW))
            ident_t = constp.tile([128, 128], bf16, tag="ident")
            nc.sync.dma_start(ident_t[:], ident_d[:])
            ones128 = constp.tile([128, 1], f32, tag="ones128")
            nc.vector.memset(ones128[:], 1.0)
            onesT = constp.tile([1, 128], f32, tag="onesT")
            nc.vector.memset(onesT[:], 1.0)
            alpha1_t = constp.tile([128, nch, 8], bf16, tag="alpha1")
            nc.sync.dma_start(alpha1_t[:], alpha1_d[:].rearrange("p (c d) -> p c d", c=nch))
            # resident W1 rhs (small; needed per destination window in layer 1)
            rt1 = constp.tile([128, 2, 3584], bf16, tag="rt1")
            nc.sync.dma_start(
                rt1[:], rhs_d[0][:, 0:3584].rearrange("(k p) n -> p k n", p=128))

            yT = None  # produced by each layer's LN for the next layer
            pending_biasT = []  # deferred biasT chunk builders for the next dense

            biasT = None  # [128, fout+2H] f32: rinv-independent LN row consts for this dense
            br = None     # [128, 2] f32: (mu, rinv) of previous layer's graph-LN

            for li in range(4):
                H, C, fin, fout = HEADS_L[li], C_L[li], FIN_L[li], FOUT_L[li]
                tcol = TCOL_L[li]
                kch = fin // 128
                acols = 2 * H

                brow = constp.tile([1, 3584], bf16, tag="brow")
                nc.sync.dma_start(brow[:, :fout], brow_d[li][:])

                if li > 0:
                    # ===== dense: h = y @ W^T (+ al columns); al chunk FIRST so the
                    # small al AllGather + softmax pre-phase overlap the dense phase
                    half = [None, 2048, 1024, 512][li]
                    shard_a = dram.tile([NPAD, half], bf16, tag=f"sharda{li}")
                    shard_b = dram.tile([NPAD, tcol - half], bf16, tag=f"shardb{li}")
                    glob_a = dram.tile([N_CORES * NPAD, half], bf16, addr_space="Shared", tag=f"globa{li}")
                    glob_b = dram.tile([N_CORES * NPAD, tcol - half], bf16, addr_space="Shared", tag=f"globb{li}")
                    ashard = dram.tile([NPAD, 64], f32, tag=f"ashard{li}")
                    aglob = dram.tile([N_CORES * NPAD, 64], f32, addr_space="Shared", tag=f"aglob{li}")
                    fcs = [(fout, acols)]
                    o = 0
                    while o < fout:
                        w_ = min(512, fout - o)
                        fcs.append((o, w_))
                        o += w_
                    # fci index after which all shard_a columns are written
                    fci_a_done = 0
                    o = 0
                    for fci, (fo, fw) in enumerate(fcs):
                        if fci > 0 and fo + fw <= half:
                            fci_a_done = fci
                    with tc.tile_pool(name=f"psA{li}", bufs=4, space="PSUM") as mmp:
                        # ---- softmax pre-phase, emitted as pipelined stages
                        # interleaved between dense row-tiles so PE/vector/
                        # scalar work overlaps the dense instead of trailing it
                        al_f32 = aglob[:]
                        alpha_all = ephp.tile([128, nch, 8], bf16, tag="alpha", bufs=1)
                        wstate = {}

                        def s1a(w):
                            c0, ncwW = coff[w], ncw[w]
                            ne = ncwW * 128
                            As = ephp.tile([128, ncmax, 64], f32, tag="as", bufs=2)
                            nc.gpsimd.dma_gather(
                                As[:, :ncwW, :], al_f32,
                                idxs_t[:, c0:c0 + ncwW, :], ne, ne, 64, elem_step=64,
                                queue_num=(2 * w) % 4)
                            maskw = mwp.tile([128, ncmax, 128], bf16, tag="mw", bufs=2)
                            nc.scalar.dma_start(
                                maskw[:, :ncwW, :],
                                mask_d[:, c0 * 128:(c0 + ncwW) * 128].rearrange(
                                    "p (c d) -> p c d", c=ncwW))
                            maskTw = mwp.tile([128, ncmax, 128], bf16, tag="mwT", bufs=2)
                            nc.scalar.dma_start(
                                maskTw[:, :ncwW, :],
                                maskT_d[:, c0 * 128:(c0 + ncwW) * 128].rearrange(
                                    "p (c d) -> p c d", c=ncwW))
                            # ald of the window's own dst nodes, scattered to edge
                            # slots via the one-hot maskT (replaces a dma_gather)
                            ald_f = ephp.tile([128, 8], f32, tag="aldf", bufs=3)
                            nc.sync.dma_start(
                                ald_f[:, :H], ashard[w * 128:(w + 1) * 128, H:2 * H])
                            ald_b = ephp.tile([128, 8], bf16, tag="aldb", bufs=3)
                            nc.vector.tensor_copy(ald_b[:, :H], ald_f[:, :H])
                            wstate[w] = (As, maskw, maskTw, ald_b)

                        def s1b(w):
                            c0, ncwW = coff[w], ncw[w]
                            As, maskw, maskTw, ald_b = wstate[w]
                            ps_ad = mmp.tile([128, ncmax, 8], f32, tag="smAd", bufs=2)
                            for c in range(ncwW):
                                nc.tensor.matmul(
                                    ps_ad[:, c, :H], maskTw[:, c, :], ald_b[:, :H],
                                    start=True, stop=True)
                            ev = ephp.tile([128, ncmax, 8], f32, tag="ev", bufs=2)
                            nc.vector.tensor_tensor(
                                ev[:, :ncwW, :H], As[:, :ncwW, 0:H], ps_ad[:, :ncwW, :H], ALU.add)
                            nc.vector.scalar_tensor_tensor(
                                ev[:, :ncwW, :H], ev[:, :ncwW, :H], 0.2, ev[:, :ncwW, :H],
                                ALU.mult, ALU.max)
                            nc.vector.tensor_scalar_min(ev[:, :ncwW, :H], ev[:, :ncwW, :H], EXP_CLAMP)
                            wv = ephp.tile([128, ncmax, 8], bf16, tag="wv", bufs=2)
                            nc.scalar.activation(wv[:, :ncwW, :H], ev[:, :ncwW, :H], ACTF.Exp)
                            wstate[w] = (wv, maskw, maskTw)

                        def s2(w):
                            c0, ncwW = coff[w], ncw[w]
                            wv, maskw, maskTw = wstate[w]
                            ps_den = mmp.tile([128, 8], f32, tag="smA", bufs=2)
                            for c in range(ncwW):
                                nc.tensor.matmul(
                                    ps_den[:, :H], maskw[:, c, :], wv[:, c, :H],
                                    start=(c == 0), stop=(c == ncwW - 1))
                            rden_f = ephp.tile([128, 8], f32, tag="rdenf", bufs=2)
                            nc.vector.tensor_scalar_max(rden_f[:, :H], ps_den[:, :H], DEN_TINY)
                            rden2 = ephp.tile([128, 8], f32, tag="rden2", bufs=2)
                            nc.vector.reciprocal(rden2[:, :H], rden_f[:, :H])
                            rden = ephp.tile([128, 8], bf16, tag="rden", bufs=2)
                            nc.vector.tensor_copy(rden[:, :H], rden2[:, :H])
                            wstate[w] = (wv, maskw, maskTw, rden)

                        def s3(w):
                            c0, ncwW = coff[w], ncw[w]
                            wv, maskw, maskTw, rden = wstate.pop(w)
                            # batched: all chunk matmuls into one PSUM tile, then a
                            # single vector mult -- no PE<->vector ping-pong stalls
                            ps_exp = mmp.tile([128, ncmax, 8], f32, tag="smAd", bufs=2)
                            for c in range(ncwW):
                                nc.tensor.matmul(
                                    ps_exp[:, c, :H], maskTw[:, c, :], rden[:, :H],
                                    start=True, stop=True)
                            nc.vector.tensor_tensor(
                                alpha_all[:, c0:c0 + ncwW, :H], wv[:, :ncwW, :H],
                                ps_exp[:, :ncwW, :H], ALU.mult)

                        qa = [(s1a, w) for w in range(NW)]
                        qb = [(f, w) for w in range(NW) for f in (s1b, s2, s3)]
                        na = nb = 0

                        def pop_stage():
                            nonlocal na, nb
                            if qa and na < nb // 3 + 3:
                                f, w = qa.pop(0); na += 1
                            elif qb and (nb // 3 + 2 <= na or not qa):
                                f, w = qb.pop(0); nb += 1
                            elif qa:
                                f, w = qa.pop(0); na += 1
                            else:
                                return False
                            f(w)
                            return True

                        per_slot = 2 if li == 3 else 1
                        slot = 0
                        for fci, (fo, fw) in enumerate(fcs):
                            kgrps = [(k0, min(7, kch - k0)) for k0 in range(0, kch, 7)]
                            rts = []
                            for (k0, kn) in kgrps:
                                rt = rhsp.tile([128, 7, 512], bf16, tag="rhs")
                                nc.sync.dma_start(
                                    rt[:, :kn, :fw],
                                    rhs_d[li][k0 * 128:(k0 + kn) * 128, fo:fo + fw]
                                    .rearrange("(k p) n -> p k n", p=128))
                                rts.append(rt)
                            for t in range(NT):
                                # deferred biasT chunks first: the eviction below
                                # reads biasT, so its writers must precede it
                                if fci >= 1:
                                    for _ in range(2):
                                        if pending_biasT:
                                            pending_biasT.pop(0)(mmp)
                                ps = mmp.tile([128, 512], f32, tag="mm")
                                for kc in range(kch):
                                    nc.tensor.matmul(
                                        ps[:, :fw],
                                        yT[:, kc, t * 128:(t + 1) * 128],
                                        rts[kc // 7][:, kc % 7, :fw],
                                        start=(kc == 0),
                                        stop=(kc == kch - 1))
                                hw = max(0, min(fw, fout - fo))
                                if hw > 0:
                                    # h' = rinv*(z.lnw @ W) + (lnb - mu*rinv*lnw)@W : the
                                    # table rows carry the exact LN'd h (see prep_params)
                                    st = stagep.tile([128, 512], bf16, tag="stg")
                                    nc.vector.scalar_tensor_tensor(
                                        st[:, :hw], ps[:, :hw], br[:, 1:2],
                                        biasT[:, fo:fo + hw], ALU.mult, ALU.subtract)
                                    if fo < half:
                                        nc.sync.dma_start(
                                            shard_a[t * 128:(t + 1) * 128, fo:fo + hw], st[:, :hw])
                                    else:
                                        nc.sync.dma_start(
                                            shard_b[t * 128:(t + 1) * 128, fo - half:fo - half + hw],
                                            st[:, :hw])
                                if hw < fw:
                                    a0 = fo + hw - fout
                                    sa = stagep.tile([128, 16], f32, tag="stga")
                                    nc.vector.scalar_tensor_tensor(
                                        sa[:, :fw - hw], ps[:, hw:fw], br[:, 1:2],
                                        biasT[:, fout + a0:fout + a0 + fw - hw],
                                        ALU.mult, ALU.subtract)
                                    nc.sync.dma_start(
                                        ashard[t * 128:(t + 1) * 128, a0:a0 + fw - hw],
                                        sa[:, :fw - hw])
                                if fci >= 1:
                                    if slot >= 2:
                                        for _ in range(per_slot):
                                            pop_stage()
                                    slot += 1
                            if fci == 0:
                                nc.gpsimd.collective_compute(
                                    "AllGather", ALU.bypass, ins=[ashard[:]], outs=[aglob[:]],
                                    replica_groups=RG)
                            if fci == fci_a_done:
                                # drain s1a gathers first: they must hit the gpsimd
                                # queue before the collective blocks it
                                while qa:
                                    pop_stage()
                                nc.gpsimd.collective_compute(
                                    "AllGather", ALU.bypass, ins=[shard_a[:]], outs=[glob_a[:]],
                                    replica_groups=RG)
                        while qa or qb:
                            pop_stage()

                    nc.gpsimd.collective_compute(
                        "AllGather", ALU.bypass, ins=[shard_b[:]], outs=[glob_b[:]],
                        replica_groups=RG)
                else:
                    alpha_all = alpha1_t

                # ===== edge phase: for li>0 two passes over column halves so
                # the glob_b AllGather hides under pass A's gather+scatter work
                edgeps = tc.tile_pool(name=f"psB{li}", bufs=1, space="PSUM")
                edgep = edgeps.__enter__()
                if li < 3:
                    zT = ytp.tile([128, 28, NPAD], bf16, tag="yt")
                    stats = miscp.tile([128, 2 * NW], f32, tag="stats")

                # ---- gather + scatter phase (single pass; glob_a covers cols
                # [0:half] with half > tcol/2 so the trailing AG_b is small)
                for w in range(NW):
                    c0, ncwW = coff[w], ncw[w]
                    maskw = mwp.tile([128, ncmax, 128], bf16, tag="mw", bufs=2)
                    nc.scalar.dma_start(
                        maskw[:, :ncwW, :],
                        mask_d[:, c0 * 128:(c0 + ncwW) * 128].rearrange(
                            "p (c d) -> p c d", c=ncwW))

                    if li == 0:
                        ps_agg = edgep.tile([128, 2, 8, 128], f32, tag="out")
                    else:
                        ps_out = edgep.tile([128, fout], f32, tag="out")

                    for cp in range(0, ncwW, 2):
                        cw = min(2, ncwW - cp)
                        # flat gather tiles: one shared tag for all layers/widths
                        if li == 0:
                            G = gathp.tile([128, 4096], bf16, tag="G")
                            nc.gpsimd.dma_gather(
                                G[:, :cw * 256].rearrange("p (c g) -> p c g", g=256),
                                xtab_d[:][:, 0:256],
                                idxs_t[:, c0 + cp:c0 + cp + cw, :], cw * 128, cw * 128,
                                256, elem_step=384, queue_num=(cp // 2) % 4)
                        else:
                            G = gathp.tile([128, 4096], bf16, tag="G")
                            nc.gpsimd.dma_gather(
                                G[:, :cw * half].rearrange("p (c g) -> p c g", g=half),
                                glob_a[:],
                                idxs_t[:, c0 + cp:c0 + cp + cw, :], cw * 128, cw * 128,
                                half, elem_step=half, queue_num=(cp // 2) % 4)
                            Gb = gathp.tile([128, 2048], bf16, tag="Gb")
                            nc.gpsimd.dma_gather(
                                Gb[:, :cw * (tcol - half)].rearrange("p (c g) -> p c g", g=tcol - half),
                                glob_b[:],
                                idxs_t[:, c0 + cp:c0 + cp + cw, :], cw * 128, cw * 128,
                                tcol - half, elem_step=tcol - half,
                                queue_num=(cp // 2 + 2) % 4)
                        for c in range(cp, cp + cw):
                            lhs = ephp.tile([128, 8, 128], bf16, tag="lhs")
                            nc.vector.tensor_tensor(
                                lhs[:, :H, :],
                                maskw[:, c, :].unsqueeze(1).broadcast_to([128, H, 128]),
                                alpha_all[:, c0 + c, :H].unsqueeze(2).broadcast_to([128, H, 128]),
                                ALU.mult)
                            if li == 0:
                                gof = (c - cp) * 256
                                for kc in range(2):
                                    for hg in range(2):
                                        nc.tensor.matmul(
                                            ps_agg[:, kc, hg * 4:(hg + 1) * 4, :],
                                            G[:, gof + kc * 128:gof + (kc + 1) * 128],
                                            lhs[:, hg * 4:(hg + 1) * 4, :],
                                            start=(c == 0), stop=False)
                            else:
                                o = 0
                                while o < fout:
                                    h = o // C
                                    e = min((h + 1) * C, (o // 512 + 1) * 512, fout)
                                    if o < half:
                                        e = min(e, half)
                                        gof = (c - cp) * half
                                        rhs_g = G[:, gof + o:gof + e]
                                    else:
                                        bof = (c - cp) * (tcol - half)
                                        rhs_g = Gb[:, bof + o - half:bof + e - half]
                                    nc.tensor.matmul(
                                        ps_out[:, o:e], lhs[:, h, :], rhs_g,
                                        start=(c == 0 and o % 512 == 0), stop=False)
                                    o = e

                    if li == 0:
                        # xaggT came out of the scatter matmuls already transposed
                        xs = zp.tile([128, 2, 8, 128], bf16, tag="z")
                        nc.vector.tensor_copy(xs[:], ps_agg[:])
                        ps_out = edgep.tile([128, fout], f32, tag="out")
                        for h in range(H):
                            o = h * C
                            while o < (h + 1) * C:
                                e = min((o // 512 + 1) * 512, (h + 1) * C)
                                for kc in range(2):
                                    nc.tensor.matmul(
                                        ps_out[:, o:e], xs[:, kc, h, :],
                                        rt1[:, kc, o:e],
                                        start=(kc == 0 and o % 512 == 0), stop=False)
                                o = e
                    # bias add
                    o = 0
                    while o < fout:
                        e = min(o + 512, fout)
                        nc.tensor.matmul(
                            ps_out[:, o:e], ones_t[:, w, :], brow[:, o:e],
                            start=False, stop=(e == fout))
                        o = e

                    if li < 3:
                        z = zp.tile([128, 3584], bf16, tag="z")
                        nc.scalar.activation(
                            z[:, :fout], ps_out[:, :fout], ACTF.Relu,
                            accum_out=stats[:, w:w + 1])
                        sq = zp.tile([128, 3584], bf16, tag="z")
                        nc.vector.scalar_tensor_tensor(
                            sq[:, :fout], z[:, :fout], 1.0, z[:, :fout],
                            ALU.mult, ALU.mult,
                            accum_out=stats[:, NW + w:NW + w + 1])
                        for q in range(0, fout // 128, 4):
                            qn = min(4, fout // 128 - q)
                            ps_t = edgep.tile([128, 4, 128], bf16, tag="sm")
                            for j in range(qn):
                                nc.tensor.matmul(
                                    ps_t[:, j, :], z[:, (q + j) * 128:(q + j + 1) * 128],
                                    ident_t[:], is_transpose=True,
                                    start=(j == 0), stop=(j == qn - 1))
                            nc.vector.tensor_copy(
                                zT[:, q:q + qn, w * 128:(w + 1) * 128],
                                ps_t[:, :qn, :])
                    else:
                        zf = zp.tile([128, 1024], f32, tag="z")
                        nc.vector.tensor_copy(zf[:, :fout], ps_out[:, :fout])
                        rows = min(128, NPC - w * 128)
                        nc.sync.dma_start(out_d[w * 128:w * 128 + rows, :], zf[:rows, :fout])

                # ===== graph LayerNorm + next yT
                if li < 3:
                    sdram = dram.tile([1, 64], f32, tag=f"sd{li}")
                    sglob = dram.tile([1, 64], f32, addr_space="Shared", tag=f"sg{li}")
                    ps_s = edgep.tile([1, 2 * NW], f32, tag="sm")
                    nc.tensor.matmul(ps_s[:], ones128[:], stats[:], start=True, stop=True)
                    ssum = miscp.tile([1, 4], f32, tag="ssum")
                    nc.vector.tensor_reduce(ssum[:, 0:1], ps_s[:, 0:NW], AX.X, ALU.add)
                    nc.vector.tensor_reduce(ssum[:, 1:2], ps_s[:, NW:2 * NW], AX.X, ALU.add)
                    nc.sync.dma_start(sdram[:, 0:2], ssum[:, 0:2])
                    nc.gpsimd.collective_compute(
                        "AllReduce", ALU.add, ins=[sdram[:]], outs=[sglob[:]],
                        replica_groups=RG)
                    gsum = miscp.tile([1, 8], f32, tag="gsum")
                    nc.sync.dma_start(gsum[:, 0:2], sglob[:, 0:2])
                    sc = miscp.tile([1, 8], f32, tag="sc")
                    inv_cnt = 1.0 / (float(N_NODES) * fout)
                    nc.vector.tensor_scalar_mul(sc[:, 0:2], gsum[:, 0:2], inv_cnt)
                    nc.vector.tensor_tensor(sc[:, 2:3], sc[:, 0:1], sc[:, 0:1], ALU.mult)
                    nc.vector.tensor_sub(sc[:, 3:4], sc[:, 1:2], sc[:, 2:3])
                    nc.vector.tensor_scalar_add(sc[:, 3:4], sc[:, 3:4], 1e-5)
                    nc.scalar.sqrt(sc[:, 4:5], sc[:, 3:4])
                    nc.vector.reciprocal(sc[:, 5:6], sc[:, 4:5])
                    mr = miscp.tile([1, 2], f32, tag="mr")
                    nc.vector.tensor_copy(mr[:, 0:1], sc[:, 0:1])
                    nc.vector.tensor_copy(mr[:, 1:2], sc[:, 5:6])
                    ps_b = edgep.tile([128, 2], f32, tag="sm")
                    nc.tensor.matmul(ps_b[:], onesT[:], mr[:], start=True, stop=True)
                    br = miscp.tile([128, 2], f32, tag=f"br{li}")
                    nc.vector.tensor_copy(br[:], ps_b[:])
                    c128 = miscp.tile([128, 1], f32, tag=f"c{li}")
                    nc.vector.tensor_tensor(c128[:], br[:, 0:1], br[:, 1:2], ALU.mult)
                    # next dense's rinv-independent row consts, negated:
                    # biasT = (mu*rinv)*r2 - r1  (evictions subtract it).
                    # Only the al-columns chunk is built here (the next dense's
                    # fci=0 evictions read it immediately); the rest is deferred
                    # into the next dense loop, hidden under its matmuls.
                    wnx = FOUT_L[li + 1] + 2 * HEADS_L[li + 1]
                    wnx0 = (FOUT_L[li + 1] // 256) * 256
                    biasT = miscp.tile([128, 3088], f32, tag="biasT", bufs=1)

                    def bt_chunk(o, e, pool, tag, pbufs=1, c128=c128, li=li, biasT=biasT):
                        r2row = miscp.tile([1, 256], f32, tag="r2row", bufs=2)
                        nc.sync.dma_start(r2row[:, :e - o], r2_d[li + 1][:, o:e])
                        r1row = miscp.tile([1, 256], f32, tag="r1row", bufs=2)
                        nc.sync.dma_start(r1row[:, :e - o], r1_d[li + 1][:, o:e])
                        rrow = miscp.tile([1, 256], f32, tag="rrow", bufs=2)
                        nc.vector.scalar_tensor_tensor(
                            rrow[:, :e - o], r2row[:, :e - o], c128[0:1, :],
                            r1row[:, :e - o], ALU.mult, ALU.subtract)
                        ps_bt = pool.tile([128, 256], f32, tag=tag, bufs=pbufs)
                        nc.tensor.matmul(
                            ps_bt[:, :e - o], onesT[:], rrow[:, :e - o],
                            start=True, stop=True)
                        nc.vector.tensor_copy(biasT[:, o:e], ps_bt[:, :e - o])

                    bt_chunk(wnx0, wnx, edgep, "sm")
                    o = 0
                    while o < wnx0:
                        e = min(o + 256, wnx0)
                        pending_biasT.append(
                            lambda pool, o=o, e=e: bt_chunk(o, e, pool, "smA", pbufs=2))
                        o = e
                    yT = zT
                    if debug:
                        nc.sync.dma_start(
                            dbg_zt[li][:], zT[:].rearrange("p q n -> p (q n)"))
                        nc.sync.dma_start(dbg_st[li][:], sc[:])
                edgeps.__exit__(None, None, None)

    nc.compile()
    _CACHE[key] = nc
    return nc


# ---------------------------------------------------------------- entry point
def make_in_maps(inputs):
    ncw, per_core = prep_edges(inputs["edge_index"])
    params = prep_params(inputs)
    alpha1 = prep_alpha1(per_core, ncw, params.pop("_al1"))
    in_maps = []
    for k in range(N_CORES):
        m = dict(params)
        m.update({kk: vv for kk, vv in per_core[k].items() if not kk.startswith("_")})
        m["alpha1"] = alpha1[k]
        in_maps.append(m)
    return ncw, in_maps


def kernel(**inputs):
    _install_ntff_hook()
    from concourse.bass_utils import run_bass_kernel_spmd

    ncw, in_maps = make_in_maps(inputs)
    nc = build(ncw)
    res = run_bass_kernel_spmd(nc, in_maps, core_ids=list(range(N_CORES)), trace=False)
    out = np.concatenate([res.results[k]["out"] for k in range(N_CORES)], axis=0)
    return out.astype(np.float32)



# revision 35
# speedup vs baseline: 1.1826x; 1.0216x over previous
"""GATNet (4-layer GAT, 10000 nodes / 50000 edges + self-loops) on 8 Trainium2 NeuronCores.

Self-contained: builds per-core shards on the host (edge bucketing by destination,
one-hot scatter masks, gather index tables), compiles one SPMD Bass program, runs it
on cores 0-7 via run_bass_kernel_spmd, and reassembles the full [10000, 1000] output.

Structure per layer:
  dense h = y @ W^T (bf16, attention projections folded as extra rhs columns).
  The previous layer's graph-LN is folded in: diag(lnw) is baked into the rhs
  weights on the host; the runtime rinv scalar and the (lnb - mu*rinv*lnw)@W
  row const (broadcast via a K=1 matmul) are applied at the PSUM->SBUF
  eviction on the Vector engine, so the h table holds the exact LN'd h and
  the dense consumes raw (pre-LN) zT -- no LN apply pass, no stats on the
  dense critical path.
  -> small AllGather of the per-node attention scores (al); the softmax
     pre-phase (als gathers; ald via one-hot maskT matmuls from the local al
     shard; e-values, segment denominators, reciprocals, alphas) overlaps the
     dense; the two big half-table AllGathers are emitted after it (collective
     instructions block the gpsimd queue until completion, so the pre-phase
     gathers must be issued first)
  -> gather phase: dma_gather of source h rows; one-hot scatter matmuls with
     the per-edge alpha folded into the stationary operand; bias via K=1
     matmul; ReLU eviction (ACT, fused row sums) with all PSUM->SBUF copies on
     the Vector engine; PE-transpose into feature-major for the next lhsT
  -> graph-LN stats via tiny AllReduce -> (mu, rinv) broadcast + next biasT.
Layer 1 never materializes h: by linearity sum_e alpha_e * (x W)[src_e] =
(sum_e alpha_e x[src_e]) W, so it scatters raw x rows (256 wide) and applies W1
once per destination window; its attention scores are computed exactly on the host.
"""
import sys
import types

import numpy as np
import ml_dtypes

BF16 = ml_dtypes.bfloat16

N_NODES = 10000
N_CORES = 8
NPC = 1250
NPAD = 1280
NT = 10
NW = 10
HEADS_L = [8, 8, 8, 1]
C_L = [448, 384, 256, 1000]
FIN_L = [256, 3584, 3072, 2048]
FOUT_L = [3584, 3072, 2048, 1000]
TCOL_L = [256, 3072, 2048, 1024]    # bf16 columns of the gather table (L1: raw x)
EXP_CLAMP = 35.0
DEN_TINY = 1e-30


def _install_ntff_hook():
    if "antenv.axon_hooks" in sys.modules:
        return
    try:
        import antenv
        from trn_agent_boot.trn_boot import _ntff_profile_via_ctypes
    except ImportError:
        return
    mod = types.ModuleType("antenv.axon_hooks")
    state = {"hook": None}
    mod.set_axon_ntff_profile_hook = lambda h: state.__setitem__("hook", h)
    mod.get_axon_ntff_profile_hook = lambda: state["hook"]
    sys.modules["antenv.axon_hooks"] = mod
    antenv.axon_hooks = mod
    mod.set_axon_ntff_profile_hook(_ntff_profile_via_ctypes("/opt/axon/libaxon_pjrt.so"))


# ---------------------------------------------------------------- host prep
def _table_row(n):
    return NPAD * (n // NPC) + (n % NPC)


def _wrap16(idx_chunk):
    w = idx_chunk.reshape(8, 16).T
    return np.tile(w, (8, 1)).astype(np.int16)


def prep_edges(edge_index):
    src = np.asarray(edge_index[0], dtype=np.int64)
    dst = np.asarray(edge_index[1], dtype=np.int64)
    src = np.concatenate([src, np.arange(N_NODES, dtype=np.int64)])
    dst = np.concatenate([dst, np.arange(N_NODES, dtype=np.int64)])

    buckets = [[[] for _ in range(NW)] for _ in range(N_CORES)]
    core_of = dst // NPC
    win_of = (dst % NPC) // 128
    order = np.argsort(dst, kind="stable")
    for e in order:
        buckets[core_of[e]][win_of[e]].append(e)

    ncw = []
    for w in range(NW):
        mx = max(len(buckets[k][w]) for k in range(N_CORES))
        ncw.append(max(1, -(-mx // 128)))
    nch = sum(ncw)

    per_core = []
    for k in range(N_CORES):
        idx_s = np.zeros((128, nch * 8), np.int16)
        idx_d = np.zeros((128, nch * 8), np.int16)
        mask = np.zeros((128, nch, 128), np.float32)
        maskT = np.zeros((128, nch, 128), np.float32)
        esrc = np.zeros((nch, 128), np.int64)
        edst = np.zeros((nch, 128), np.int64)
        ereal = np.zeros((nch, 128), bool)
        c0 = 0
        for w in range(NW):
            edges = buckets[k][w]
            for c in range(ncw[w]):
                part = edges[c * 128:(c + 1) * 128]
                srows = np.zeros(128, np.int64)
                drows = np.zeros(128, np.int64)
                for i, e in enumerate(part):
                    srows[i] = _table_row(src[e])
                    drows[i] = _table_row(dst[e])
                    esrc[c0 + c, i] = src[e]
                    edst[c0 + c, i] = dst[e]
                    ereal[c0 + c, i] = True
                    d_local = (dst[e] % NPC) - 128 * w
                    mask[i, c0 + c, d_local] = 1.0
                    maskT[d_local, c0 + c, i] = 1.0
                idx_s[:, (c0 + c) * 8:(c0 + c + 1) * 8] = _wrap16(srows)
                idx_d[:, (c0 + c) * 8:(c0 + c + 1) * 8] = _wrap16(drows)
            c0 += ncw[w]
        per_core.append(dict(
            idxs=idx_s,
            mask=mask.reshape(128, nch * 128).astype(BF16),
            maskT=maskT.reshape(128, nch * 128).astype(BF16),
            _esrc=esrc, _edst=edst, _ereal=ereal,
        ))
    return tuple(ncw), per_core


def prep_alpha1(per_core, ncw, al1):
    """Exact layer-1 softmax on the host: alpha[e, h] per (chunk, slot)."""
    nch = sum(ncw)
    als = al1[:, :8].astype(np.float64)
    ald = al1[:, 8:].astype(np.float64)
    out = []
    for pc in per_core:
        esrc, edst, ereal = pc["_esrc"], pc["_edst"], pc["_ereal"]
        e = als[esrc] + ald[edst]                      # [nch, 128, 8]
        e = np.maximum(e, 0.2 * e)
        wv = np.exp(np.minimum(e, EXP_CLAMP)) * ereal[:, :, None]
        den = np.zeros((NPC, 8))
        np.add.at(den, (edst % NPC).reshape(-1), wv.reshape(-1, 8))
        alpha = wv / np.maximum(den[(edst % NPC)], 1e-300)
        # device layout: [part=slot, nch*8]
        return_arr = np.ascontiguousarray(
            alpha.transpose(1, 0, 2).reshape(128, nch * 8)).astype(BF16)
        out.append(return_arr)
    return out


def prep_params(inputs):
    p = {}
    x64 = np.asarray(inputs["x"], np.float64)
    al1 = None
    for li in range(4):
        H, C, fin, fout = HEADS_L[li], C_L[li], FIN_L[li], FOUT_L[li]
        W = np.asarray(inputs[f"W{li+1}"], np.float64)
        a_src = np.asarray(inputs[f"a_src{li+1}"], np.float64)
        a_dst = np.asarray(inputs[f"a_dst{li+1}"], np.float64)
        a_blk_s = np.zeros((fout, H), np.float64)
        a_blk_d = np.zeros((fout, H), np.float64)
        for h in range(H):
            a_blk_s[h * C:(h + 1) * C, h] = a_src[h]
            a_blk_d[h * C:(h + 1) * C, h] = a_dst[h]
        rhs = np.concatenate([W.T, W.T @ a_blk_s, W.T @ a_blk_d], axis=1)
        # graph-LN of the previous layer folded in: y = rinv*(z*lnw) + (lnb - mu*rinv*lnw)
        # so h = y@rhs = rinv*(z @ diag(lnw)@rhs) + r1 - (mu*rinv)*r2 with the
        # static diag(lnw) baked into the device rhs and r1/r2 host rows.
        if li > 0:
            lwp = np.asarray(inputs[f"ln{li}_w"], np.float64)
            lbp = np.asarray(inputs[f"ln{li}_b"], np.float64)
            p[f"rhs{li+1}"] = np.ascontiguousarray(lwp[:, None] * rhs).astype(BF16)
            p[f"r1_{li+1}"] = (lbp @ rhs).reshape(1, fout + 2 * H).astype(np.float32)
            p[f"r2_{li+1}"] = (lwp @ rhs).reshape(1, fout + 2 * H).astype(np.float32)
        else:
            p[f"rhs{li+1}"] = np.ascontiguousarray(rhs).astype(BF16)
        p[f"brow{li+1}"] = np.asarray(inputs[f"b{li+1}"], np.float32).reshape(1, fout).astype(BF16)
        if li == 0:
            al1 = np.concatenate(
                [x64 @ (W.T @ a_blk_s), x64 @ (W.T @ a_blk_d)], axis=1).astype(np.float32)
    # packed x table, replicated: [10240 rows, 768 bytes] = 512B x bf16 + 256B f32 al
    xtab = np.zeros((N_CORES * NPAD, 768), np.uint8)
    rows = _table_row(np.arange(N_NODES))
    xb = np.asarray(inputs["x"], np.float32).astype(BF16)
    xtab[rows, :512] = xb.view(np.uint8)
    alpad = np.zeros((N_NODES, 64), np.float32)
    alpad[:, :16] = al1
    xtab[rows, 512:768] = alpad.view(np.uint8)
    p["xtab"] = xtab.view(BF16)
    p["_al1"] = al1
    p["ident"] = np.eye(128, dtype=BF16)
    ones_b = np.zeros((1, NW * 128), np.float32)
    ones_b[0, :NPC] = 1.0
    p["ones_b"] = ones_b.astype(BF16)
    return p


# ---------------------------------------------------------------- device build
_CACHE = {}


def build(ncw, debug=False):
    key = (tuple(ncw), debug)
    if key in _CACHE:
        return _CACHE[key]

    import concourse.bacc as bacc
    import concourse.mybir as mybir
    import concourse.tile as tile
    from concourse.library_config import mlp

    f32 = mybir.dt.float32
    bf16 = mybir.dt.bfloat16
    i16 = mybir.dt.int16
    AX = mybir.AxisListType
    ALU = mybir.AluOpType
    ACTF = mybir.ActivationFunctionType

    nch = sum(ncw)
    ncmax = max(ncw)
    coff = [0]
    for w in range(NW):
        coff.append(coff[-1] + ncw[w])
    nc = bacc.Bacc("TRN2", num_swdge_queues=4)

    xtab_d = nc.declare_dram_parameter("xtab", [N_CORES * NPAD, 384], bf16, isOutput=False)
    rhs_d, brow_d, r1_d, r2_d = [], [], [None], [None]
    for li in range(4):
        H, fout, fin = HEADS_L[li], FOUT_L[li], FIN_L[li]
        rhs_d.append(nc.declare_dram_parameter(f"rhs{li+1}", [fin, fout + 2 * H], bf16, isOutput=False))
        brow_d.append(nc.declare_dram_parameter(f"brow{li+1}", [1, fout], bf16, isOutput=False))
        if li > 0:
            r1_d.append(nc.declare_dram_parameter(f"r1_{li+1}", [1, fout + 2 * H], f32, isOutput=False))
            r2_d.append(nc.declare_dram_parameter(f"r2_{li+1}", [1, fout + 2 * H], f32, isOutput=False))
    alpha1_d = nc.declare_dram_parameter("alpha1", [128, nch * 8], bf16, isOutput=False)
    idxs_d = nc.declare_dram_parameter("idxs", [128, nch * 8], i16, isOutput=False)
    mask_d = nc.declare_dram_parameter("mask", [128, nch * 128], bf16, isOutput=False)
    maskT_d = nc.declare_dram_parameter("maskT", [128, nch * 128], bf16, isOutput=False)
    ones_d = nc.declare_dram_parameter("ones_b", [1, NW * 128], bf16, isOutput=False)
    ident_d = nc.declare_dram_parameter("ident", [128, 128], bf16, isOutput=False)
    out_d = nc.declare_dram_parameter("out", [NPC, 1000], f32, isOutput=True)
    dbg_zt, dbg_st = [], []
    if debug:
        for li in range(3):
            dbg_zt.append(nc.declare_dram_parameter(
                f"dbg_zt{li}", [128, 28 * NPAD], bf16, isOutput=True))
            dbg_st.append(nc.declare_dram_parameter(
                f"dbg_st{li}", [1, 8], f32, isOutput=True))

    RG = [list(range(N_CORES))]

    with tile.TileContext(nc) as tc:
        with (
            tc.tile_pool(name="const", bufs=1) as constp,
            tc.tile_pool(name="yt", bufs=1) as ytp,
            tc.tile_pool(name="rhs", bufs=5) as rhsp,
            tc.tile_pool(name="stage", bufs=2) as stagep,
            tc.tile_pool(name="gath", bufs=2) as gathp,
            tc.tile_pool(name="mw", bufs=2) as mwp,
            tc.tile_pool(name="eph", bufs=2) as ephp,
            tc.tile_pool(name="z", bufs=2) as zp,
            tc.tile_pool(name="misc", bufs=2) as miscp,
            tc.tile_pool(name="dram", bufs=1, space="DRAM") as dram,
        ):
            nc.gpsimd.load_library(mlp)

            # warmup collective: the first CC op on the device pays a ~140us
            # one-time init; issue a tiny AllReduce up front so that cost
            # overlaps the L1 edge phase instead of the L1 LN AllReduce
            warm_sb = miscp.tile([1, 8], f32, tag="warm")
            nc.vector.memset(warm_sb[:], 1.0)
            warm_dram = dram.tile([1, 8], f32, tag="warmd")
            warm_glob = dram.tile([1, 8], f32, addr_space="Shared", tag="warmg")
            nc.sync.dma_start(warm_dram[:], warm_sb[:])
            nc.gpsimd.collective_compute(
                "AllReduce", ALU.add, ins=[warm_dram[:]], outs=[warm_glob[:]],
                replica_groups=RG)

            idxs_t = constp.tile([128, nch, 8], i16, tag="idxs")
            nc.sync.dma_start(idxs_t[:], idxs_d[:].rearrange("p (c d) -> p c d", c=nch))
            ones_t = constp.tile([1, NW, 128], bf16, tag="onesb")
            nc.sync.dma_start(ones_t[:], ones_d[:].rearrange("p (w d) -> p w d", w=NW))
            ident_t = constp.tile([128, 128], bf16, tag="ident")
            nc.sync.dma_start(ident_t[:], ident_d[:])
            ones128 = constp.tile([128, 1], f32, tag="ones128")
            nc.vector.memset(ones128[:], 1.0)
            onesT = constp.tile([1, 128], f32, tag="onesT")
            nc.vector.memset(onesT[:], 1.0)
            alpha1_t = constp.tile([128, nch, 8], bf16, tag="alpha1")
            nc.sync.dma_start(alpha1_t[:], alpha1_d[:].rearrange("p (c d) -> p c d", c=nch))
            # resident W1 rhs (small; needed per destination window in layer 1)
            rtw = max(3584, nch * 64)  # tag shared with the As_all region tile
            rt1 = constp.tile([128, 2, rtw], bf16, tag="rt1")
            nc.sync.dma_start(
                rt1[:, :, :3584],
                rhs_d[0][:, 0:3584].rearrange("(k p) n -> p k n", p=128))

            yT = None  # produced by each layer's LN for the next layer
            pending_biasT = []  # deferred biasT chunk builders for the next dense

            biasT = None  # [128, fout+2H] f32: rinv-independent LN row consts for this dense
            br = None     # [128, 2] f32: (mu, rinv) of previous layer's graph-LN

            for li in range(4):
                H, C, fin, fout = HEADS_L[li], C_L[li], FIN_L[li], FOUT_L[li]
                tcol = TCOL_L[li]
                kch = fin // 128
                acols = 2 * H

                brow = constp.tile([1, 3584], bf16, tag="brow")
                nc.sync.dma_start(brow[:, :fout], brow_d[li][:])

                if li > 0:
                    # ===== dense: h = y @ W^T (+ al columns); al chunk FIRST so the
                    # small al AllGather + softmax pre-phase overlap the dense phase
                    half = [None, 2048, 1024, 512][li]
                    shard_a = dram.tile([NPAD, half], bf16, tag=f"sharda{li}")
                    shard_b = dram.tile([NPAD, tcol - half], bf16, tag=f"shardb{li}")
                    glob_a = dram.tile([N_CORES * NPAD, half], bf16, addr_space="Shared", tag=f"globa{li}")
                    glob_b = dram.tile([N_CORES * NPAD, tcol - half], bf16, addr_space="Shared", tag=f"globb{li}")
                    ashard = dram.tile([NPAD, 64], f32, tag=f"ashard{li}")
                    aglob = dram.tile([N_CORES * NPAD, 64], f32, addr_space="Shared", tag=f"aglob{li}")
                    fcs = [(fout, acols)]
                    o = 0
                    while o < fout:
                        w_ = min(512, fout - o)
                        fcs.append((o, w_))
                        o += w_
                    # fci index after which all shard_a columns are written
                    fci_a_done = 0
                    o = 0
                    for fci, (fo, fw) in enumerate(fcs):
                        if fci > 0 and fo + fw <= half:
                            fci_a_done = fci
                    with tc.tile_pool(name=f"psA{li}", bufs=4, space="PSUM") as mmp:
                        # ---- softmax pre-phase, emitted as pipelined stages
                        # interleaved between dense row-tiles so PE/vector/
                        # scalar work overlaps the dense instead of trailing it
                        al_f32 = aglob[:]
                        alpha_all = ephp.tile([128, nch, 8], bf16, tag="alpha", bufs=1)
                        wstate = {}
                        As_all = constp.tile([128, nch, 64], f32, tag="rt1")

                        def s1a(w):
                            c0, ncwW = coff[w], ncw[w]
                            ne = ncwW * 128
                            As = As_all[:, c0:c0 + ncwW, :]
                            nc.gpsimd.dma_gather(
                                As, al_f32,
                                idxs_t[:, c0:c0 + ncwW, :], ne, ne, 64, elem_step=64,
                                queue_num=(2 * w) % 4)
                            maskw = mwp.tile([128, ncmax, 128], bf16, tag="mw", bufs=2)
                            nc.scalar.dma_start(
                                maskw[:, :ncwW, :],
                                mask_d[:, c0 * 128:(c0 + ncwW) * 128].rearrange(
                                    "p (c d) -> p c d", c=ncwW))
                            maskTw = mwp.tile([128, ncmax, 128], bf16, tag="mwT", bufs=2)
                            nc.scalar.dma_start(
                                maskTw[:, :ncwW, :],
                                maskT_d[:, c0 * 128:(c0 + ncwW) * 128].rearrange(
                                    "p (c d) -> p c d", c=ncwW))
                            # ald of the window's own dst nodes, scattered to edge
                            # slots via the one-hot maskT (replaces a dma_gather)
                            ald_f = ephp.tile([128, 8], f32, tag="aldf", bufs=3)
                            nc.sync.dma_start(
                                ald_f[:, :H], ashard[w * 128:(w + 1) * 128, H:2 * H])
                            ald_b = ephp.tile([128, 8], bf16, tag="aldb", bufs=3)
                            nc.vector.tensor_copy(ald_b[:, :H], ald_f[:, :H])
                            wstate[w] = (As, maskw, maskTw, ald_b)

                        def s1b(w):
                            c0, ncwW = coff[w], ncw[w]
                            As, maskw, maskTw, ald_b = wstate[w]
                            ps_ad = mmp.tile([128, ncmax, 8], f32, tag="smAd", bufs=2)
                            for c in range(ncwW):
                                nc.tensor.matmul(
                                    ps_ad[:, c, :H], maskTw[:, c, :], ald_b[:, :H],
                                    start=True, stop=True)
                            ev = ephp.tile([128, ncmax, 8], f32, tag="ev", bufs=1)
                            nc.vector.tensor_tensor(
                                ev[:, :ncwW, :H], As[:, :, 0:H], ps_ad[:, :ncwW, :H], ALU.add)
                            nc.vector.scalar_tensor_tensor(
                                ev[:, :ncwW, :H], ev[:, :ncwW, :H], 0.2, ev[:, :ncwW, :H],
                                ALU.mult, ALU.max)
                            nc.vector.tensor_scalar_min(ev[:, :ncwW, :H], ev[:, :ncwW, :H], EXP_CLAMP)
                            wv = ephp.tile([128, ncmax, 8], bf16, tag="wv", bufs=2)
                            nc.scalar.activation(wv[:, :ncwW, :H], ev[:, :ncwW, :H], ACTF.Exp)
                            wstate[w] = (wv, maskw, maskTw)

                        def s2(w):
                            c0, ncwW = coff[w], ncw[w]
                            wv, maskw, maskTw = wstate[w]
                            ps_den = mmp.tile([128, 8], f32, tag="smA", bufs=2)
                            for c in range(ncwW):
                                nc.tensor.matmul(
                                    ps_den[:, :H], maskw[:, c, :], wv[:, c, :H],
                                    start=(c == 0), stop=(c == ncwW - 1))
                            rden_f = ephp.tile([128, 8], f32, tag="rdenf", bufs=2)
                            nc.vector.tensor_scalar_max(rden_f[:, :H], ps_den[:, :H], DEN_TINY)
                            rden2 = ephp.tile([128, 8], f32, tag="rden2", bufs=2)
                            nc.vector.reciprocal(rden2[:, :H], rden_f[:, :H])
                            rden = ephp.tile([128, 8], bf16, tag="rden", bufs=2)
                            nc.vector.tensor_copy(rden[:, :H], rden2[:, :H])
                            wstate[w] = (wv, maskw, maskTw, rden)

                        def s3(w):
                            c0, ncwW = coff[w], ncw[w]
                            wv, maskw, maskTw, rden = wstate.pop(w)
                            # batched: all chunk matmuls into one PSUM tile, then a
                            # single vector mult -- no PE<->vector ping-pong stalls
                            ps_exp = mmp.tile([128, ncmax, 8], f32, tag="smAd", bufs=2)
                            for c in range(ncwW):
                                nc.tensor.matmul(
                                    ps_exp[:, c, :H], maskTw[:, c, :], rden[:, :H],
                                    start=True, stop=True)
                            nc.vector.tensor_tensor(
                                alpha_all[:, c0:c0 + ncwW, :H], wv[:, :ncwW, :H],
                                ps_exp[:, :ncwW, :H], ALU.mult)

                        qa = [(s1a, w) for w in range(NW)]
                        qb = [(f, w) for w in range(NW) for f in (s1b, s2, s3)]
                        na = nb = 0

                        def pop_stage():
                            nonlocal na, nb
                            if qa and na < nb // 3 + 3:
                                f, w = qa.pop(0); na += 1
                            elif qb and (nb // 3 + 2 <= na or not qa):
                                f, w = qb.pop(0); nb += 1
                            elif qa:
                                f, w = qa.pop(0); na += 1
                            else:
                                return False
                            f(w)
                            return True

                        per_slot = 2 if li == 3 else 1
                        slot = 0
                        for fci, (fo, fw) in enumerate(fcs):
                            kgrps = [(k0, min(7, kch - k0)) for k0 in range(0, kch, 7)]
                            rts = []
                            for (k0, kn) in kgrps:
                                rt = rhsp.tile([128, 7, 512], bf16, tag="rhs")
                                nc.sync.dma_start(
                                    rt[:, :kn, :fw],
                                    rhs_d[li][k0 * 128:(k0 + kn) * 128, fo:fo + fw]
                                    .rearrange("(k p) n -> p k n", p=128))
                                rts.append(rt)
                            for t in range(NT):
                                # deferred biasT chunks first: the eviction below
                                # reads biasT, so its writers must precede it
                                if fci >= 1:
                                    for _ in range(2):
                                        if pending_biasT:
                                            pending_biasT.pop(0)(mmp)
                                ps = mmp.tile([128, 512], f32, tag="mm")
                                for kc in range(kch):
                                    nc.tensor.matmul(
                                        ps[:, :fw],
                                        yT[:, kc, t * 128:(t + 1) * 128],
                                        rts[kc // 7][:, kc % 7, :fw],
                                        start=(kc == 0),
                                        stop=(kc == kch - 1))
                                hw = max(0, min(fw, fout - fo))
                                if hw > 0:
                                    # h' = rinv*(z.lnw @ W) + (lnb - mu*rinv*lnw)@W : the
                                    # table rows carry the exact LN'd h (see prep_params)
                                    st = stagep.tile([128, 512], bf16, tag="stg", bufs=3)
                                    nc.vector.scalar_tensor_tensor(
                                        st[:, :hw], ps[:, :hw], br[:, 1:2],
                                        biasT[:, fo:fo + hw], ALU.mult, ALU.subtract)
                                    if fo < half:
                                        nc.sync.dma_start(
                                            shard_a[t * 128:(t + 1) * 128, fo:fo + hw], st[:, :hw])
                                    else:
                                        nc.sync.dma_start(
                                            shard_b[t * 128:(t + 1) * 128, fo - half:fo - half + hw],
                                            st[:, :hw])
                                if hw < fw:
                                    a0 = fo + hw - fout
                                    sa = stagep.tile([128, 16], f32, tag="stga")
                                    nc.vector.scalar_tensor_tensor(
                                        sa[:, :fw - hw], ps[:, hw:fw], br[:, 1:2],
                                        biasT[:, fout + a0:fout + a0 + fw - hw],
                                        ALU.mult, ALU.subtract)
                                    nc.sync.dma_start(
                                        ashard[t * 128:(t + 1) * 128, a0:a0 + fw - hw],
                                        sa[:, :fw - hw])
                                if fci >= 1:
                                    if slot >= 2:
                                        for _ in range(per_slot):
                                            pop_stage()
                                    slot += 1
                            if fci == 0:
                                nc.gpsimd.collective_compute(
                                    "AllGather", ALU.bypass, ins=[ashard[:]], outs=[aglob[:]],
                                    replica_groups=RG)
                            if fci == fci_a_done:
                                # drain s1a gathers first: they must hit the gpsimd
                                # queue before the collective blocks it
                                while qa:
                                    pop_stage()
                                nc.gpsimd.collective_compute(
                                    "AllGather", ALU.bypass, ins=[shard_a[:]], outs=[glob_a[:]],
                                    replica_groups=RG)
                        while qa or qb:
                            pop_stage()

                    nc.gpsimd.collective_compute(
                        "AllGather", ALU.bypass, ins=[shard_b[:]], outs=[glob_b[:]],
                        replica_groups=RG)
                else:
                    alpha_all = alpha1_t

                # ===== edge phase: for li>0 two passes over column halves so
                # the glob_b AllGather hides under pass A's gather+scatter work
                edgeps = tc.tile_pool(name=f"psB{li}", bufs=1, space="PSUM")
                edgep = edgeps.__enter__()
                if li < 3:
                    zT = ytp.tile([128, 28, NPAD], bf16, tag="yt")
                    stats = miscp.tile([128, 2 * NW], f32, tag="stats")

                # ---- gather + scatter phase (single pass; glob_a covers cols
                # [0:half] with half > tcol/2 so the trailing AG_b is small)
                for w in range(NW):
                    c0, ncwW = coff[w], ncw[w]
                    maskw = mwp.tile([128, ncmax, 128], bf16, tag="mw", bufs=2)
                    nc.scalar.dma_start(
                        maskw[:, :ncwW, :],
                        mask_d[:, c0 * 128:(c0 + ncwW) * 128].rearrange(
                            "p (c d) -> p c d", c=ncwW))

                    if li == 0:
                        ps_agg = edgep.tile([128, 2, 8, 128], f32, tag="out")
                    else:
                        ps_out = edgep.tile([128, fout], f32, tag="out")

                    for cp in range(0, ncwW, 2):
                        cw = min(2, ncwW - cp)
                        # flat gather tiles: one shared tag for all layers/widths
                        if li == 0:
                            G = gathp.tile([128, 4096], bf16, tag="G")
                            nc.gpsimd.dma_gather(
                                G[:, :cw * 256].rearrange("p (c g) -> p c g", g=256),
                                xtab_d[:][:, 0:256],
                                idxs_t[:, c0 + cp:c0 + cp + cw, :], cw * 128, cw * 128,
                                256, elem_step=384, queue_num=(cp // 2) % 4)
                        else:
                            G = gathp.tile([128, 4096], bf16, tag="G")
                            nc.gpsimd.dma_gather(
                                G[:, :cw * half].rearrange("p (c g) -> p c g", g=half),
                                glob_a[:],
                                idxs_t[:, c0 + cp:c0 + cp + cw, :], cw * 128, cw * 128,
                                half, elem_step=half, queue_num=(cp // 2) % 4)
                            Gb = gathp.tile([128, 2048], bf16, tag="Gb")
                            nc.gpsimd.dma_gather(
                                Gb[:, :cw * (tcol - half)].rearrange("p (c g) -> p c g", g=tcol - half),
                                glob_b[:],
                                idxs_t[:, c0 + cp:c0 + cp + cw, :], cw * 128, cw * 128,
                                tcol - half, elem_step=tcol - half,
                                queue_num=(cp // 2 + 2) % 4)
                        for c in range(cp, cp + cw):
                            lhs = ephp.tile([128, 8, 128], bf16, tag="lhs")
                            nc.vector.tensor_tensor(
                                lhs[:, :H, :],
                                maskw[:, c, :].unsqueeze(1).broadcast_to([128, H, 128]),
                                alpha_all[:, c0 + c, :H].unsqueeze(2).broadcast_to([128, H, 128]),
                                ALU.mult)
                            if li == 0:
                                gof = (c - cp) * 256
                                for kc in range(2):
                                    for hg in range(2):
                                        nc.tensor.matmul(
                                            ps_agg[:, kc, hg * 4:(hg + 1) * 4, :],
                                            G[:, gof + kc * 128:gof + (kc + 1) * 128],
                                            lhs[:, hg * 4:(hg + 1) * 4, :],
                                            start=(c == 0), stop=False)
                            else:
                                o = 0
                                while o < fout:
                                    h = o // C
                                    e = min((h + 1) * C, (o // 512 + 1) * 512, fout)
                                    if o < half:
                                        e = min(e, half)
                                        gof = (c - cp) * half
                                        rhs_g = G[:, gof + o:gof + e]
                                    else:
                                        bof = (c - cp) * (tcol - half)
                                        rhs_g = Gb[:, bof + o - half:bof + e - half]
                                    nc.tensor.matmul(
                                        ps_out[:, o:e], lhs[:, h, :], rhs_g,
                                        start=(c == 0 and o % 512 == 0), stop=False)
                                    o = e

                    if li == 0:
                        # xaggT came out of the scatter matmuls already transposed
                        xs = zp.tile([128, 2, 8, 128], bf16, tag="z")
                        nc.vector.tensor_copy(xs[:], ps_agg[:])
                        ps_out = edgep.tile([128, fout], f32, tag="out")
                        for h in range(H):
                            o = h * C
                            while o < (h + 1) * C:
                                e = min((o // 512 + 1) * 512, (h + 1) * C)
                                for kc in range(2):
                                    nc.tensor.matmul(
                                        ps_out[:, o:e], xs[:, kc, h, :],
                                        rt1[:, kc, o:e],
                                        start=(kc == 0 and o % 512 == 0), stop=False)
                                o = e
                    # bias add
                    o = 0
                    while o < fout:
                        e = min(o + 512, fout)
                        nc.tensor.matmul(
                            ps_out[:, o:e], ones_t[:, w, :], brow[:, o:e],
                            start=False, stop=(e == fout))
                        o = e

                    if li < 3:
                        z = zp.tile([128, 3584], bf16, tag="z")
                        nc.scalar.activation(
                            z[:, :fout], ps_out[:, :fout], ACTF.Relu,
                            accum_out=stats[:, w:w + 1])
                        sq = zp.tile([128, 3584], bf16, tag="z")
                        nc.vector.scalar_tensor_tensor(
                            sq[:, :fout], z[:, :fout], 1.0, z[:, :fout],
                            ALU.mult, ALU.mult,
                            accum_out=stats[:, NW + w:NW + w + 1])
                        for q in range(0, fout // 128, 4):
                            qn = min(4, fout // 128 - q)
                            ps_t = edgep.tile([128, 4, 128], bf16, tag="sm")
                            for j in range(qn):
                                nc.tensor.matmul(
                                    ps_t[:, j, :], z[:, (q + j) * 128:(q + j + 1) * 128],
                                    ident_t[:], is_transpose=True,
                                    start=(j == 0), stop=(j == qn - 1))
                            nc.vector.tensor_copy(
                                zT[:, q:q + qn, w * 128:(w + 1) * 128],
                                ps_t[:, :qn, :])
                    else:
                        zf = zp.tile([128, 1024], f32, tag="z")
                        nc.vector.tensor_copy(zf[:, :fout], ps_out[:, :fout])
                        rows = min(128, NPC - w * 128)
                        nc.sync.dma_start(out_d[w * 128:w * 128 + rows, :], zf[:rows, :fout])

                # ===== graph LayerNorm + next yT
                if li < 3:
                    sdram = dram.tile([1, 64], f32, tag=f"sd{li}")
                    sglob = dram.tile([1, 64], f32, addr_space="Shared", tag=f"sg{li}")
                    ps_s = edgep.tile([1, 2 * NW], f32, tag="sm")
                    nc.tensor.matmul(ps_s[:], ones128[:], stats[:], start=True, stop=True)
                    ssum = miscp.tile([1, 4], f32, tag="ssum")
                    nc.vector.tensor_reduce(ssum[:, 0:1], ps_s[:, 0:NW], AX.X, ALU.add)
                    nc.vector.tensor_reduce(ssum[:, 1:2], ps_s[:, NW:2 * NW], AX.X, ALU.add)
                    nc.sync.dma_start(sdram[:, 0:2], ssum[:, 0:2])
                    nc.gpsimd.collective_compute(
                        "AllReduce", ALU.add, ins=[sdram[:]], outs=[sglob[:]],
                        replica_groups=RG)
                    gsum = miscp.tile([1, 8], f32, tag="gsum")
                    nc.sync.dma_start(gsum[:, 0:2], sglob[:, 0:2])
                    sc = miscp.tile([1, 8], f32, tag="sc")
                    inv_cnt = 1.0 / (float(N_NODES) * fout)
                    nc.vector.tensor_scalar_mul(sc[:, 0:2], gsum[:, 0:2], inv_cnt)
                    nc.vector.tensor_tensor(sc[:, 2:3], sc[:, 0:1], sc[:, 0:1], ALU.mult)
                    nc.vector.tensor_sub(sc[:, 3:4], sc[:, 1:2], sc[:, 2:3])
                    nc.vector.tensor_scalar_add(sc[:, 3:4], sc[:, 3:4], 1e-5)
                    nc.scalar.sqrt(sc[:, 4:5], sc[:, 3:4])
                    nc.vector.reciprocal(sc[:, 5:6], sc[:, 4:5])
                    mr = miscp.tile([1, 2], f32, tag="mr")
                    nc.vector.tensor_copy(mr[:, 0:1], sc[:, 0:1])
                    nc.vector.tensor_copy(mr[:, 1:2], sc[:, 5:6])
                    ps_b = edgep.tile([128, 2], f32, tag="sm")
                    nc.tensor.matmul(ps_b[:], onesT[:], mr[:], start=True, stop=True)
                    br = miscp.tile([128, 2], f32, tag=f"br{li}")
                    nc.vector.tensor_copy(br[:], ps_b[:])
                    c128 = miscp.tile([128, 1], f32, tag=f"c{li}")
                    nc.vector.tensor_tensor(c128[:], br[:, 0:1], br[:, 1:2], ALU.mult)
                    # next dense's rinv-independent row consts, negated:
                    # biasT = (mu*rinv)*r2 - r1  (evictions subtract it).
                    # Only the al-columns chunk is built here (the next dense's
                    # fci=0 evictions read it immediately); the rest is deferred
                    # into the next dense loop, hidden under its matmuls.
                    wnx = FOUT_L[li + 1] + 2 * HEADS_L[li + 1]
                    wnx0 = (FOUT_L[li + 1] // 256) * 256
                    biasT = miscp.tile([128, 3088], f32, tag="biasT", bufs=1)

                    def bt_chunk(o, e, pool, tag, pbufs=1, c128=c128, li=li, biasT=biasT):
                        r2row = miscp.tile([1, 256], f32, tag="r2row", bufs=2)
                        nc.sync.dma_start(r2row[:, :e - o], r2_d[li + 1][:, o:e])
                        r1row = miscp.tile([1, 256], f32, tag="r1row", bufs=2)
                        nc.sync.dma_start(r1row[:, :e - o], r1_d[li + 1][:, o:e])
                        rrow = miscp.tile([1, 256], f32, tag="rrow", bufs=2)
                        nc.vector.scalar_tensor_tensor(
                            rrow[:, :e - o], r2row[:, :e - o], c128[0:1, :],
                            r1row[:, :e - o], ALU.mult, ALU.subtract)
                        ps_bt = pool.tile([128, 256], f32, tag=tag, bufs=pbufs)
                        nc.tensor.matmul(
                            ps_bt[:, :e - o], onesT[:], rrow[:, :e - o],
                            start=True, stop=True)
                        nc.vector.tensor_copy(biasT[:, o:e], ps_bt[:, :e - o])

                    bt_chunk(wnx0, wnx, edgep, "sm")
                    o = 0
                    while o < wnx0:
                        e = min(o + 256, wnx0)
                        pending_biasT.append(
                            lambda pool, o=o, e=e: bt_chunk(o, e, pool, "smA", pbufs=2))
                        o = e
                    yT = zT
                    if debug:
                        nc.sync.dma_start(
                            dbg_zt[li][:], zT[:].rearrange("p q n -> p (q n)"))
                        nc.sync.dma_start(dbg_st[li][:], sc[:])
                edgeps.__exit__(None, None, None)

    nc.compile()
    _CACHE[key] = nc
    return nc


# ---------------------------------------------------------------- entry point
def make_in_maps(inputs):
    ncw, per_core = prep_edges(inputs["edge_index"])
    params = prep_params(inputs)
    alpha1 = prep_alpha1(per_core, ncw, params.pop("_al1"))
    in_maps = []
    for k in range(N_CORES):
        m = dict(params)
        m.update({kk: vv for kk, vv in per_core[k].items() if not kk.startswith("_")})
        m["alpha1"] = alpha1[k]
        in_maps.append(m)
    return ncw, in_maps


def kernel(**inputs):
    _install_ntff_hook()
    from concourse.bass_utils import run_bass_kernel_spmd

    ncw, in_maps = make_in_maps(inputs)
    nc = build(ncw)
    res = run_bass_kernel_spmd(nc, in_maps, core_ids=list(range(N_CORES)), trace=False)
    out = np.concatenate([res.results[k]["out"] for k in range(N_CORES)], axis=0)
    return out.astype(np.float32)

